# revision 22
# baseline (speedup 1.0000x reference)
"""Trainium2 Bass kernel for nn_DetectionLoss (B=128, N=1024, MAX_T=64, 80 classes).

Contract: kernel(**inputs) takes FULL inputs {preds: (128,1024,85) f32,
targets: (128,64,5) f32} and returns the FULL scalar output (f32 (),
mean of per-sample losses), computed data-parallel on 8 NeuronCores
(16 samples per core).

v4 design (vs v2 baseline at ~207us, v3 at ~211us):
- Grid layout [P, r, j, s] with s innermost: broadcasts are stride-0 MIDDLE
  dims, so every DVE tensor_tensor runs in 2x mode with zero replicate copies.
- Targets broadcast via 10 ones-matmuls into batched PSUM, evacuated with
  TWO large Scalar copies (v3 did 10 small ones, latency-serialized).
- Invalid targets: x1=x2=0 via vb-multiply on Pool (inter==0, never wins).
- Pair phase in 2 sample-halves; Pool computes a12 = pa+ta concurrently.
- Division: den bf16 (2x) -> Scalar upcast +1e-6 bias -> DVE
  reciprocal_approx_fast (only 1x op) -> Scalar downcast, 4-quarter pipeline.
- Gather: eq is_eq at 2x; field q=3 gathered fully on Pool, rest on DVE.
- CE: Scalar exp/label-broadcast lead DVE trees; pick multiply on Pool.
- Fallback-mask chain placed right after BEST so its PE transposes +
  Scalar copies overlap gather.
"""
import numpy as np

import concourse.bass as bass
import concourse.bacc as bacc
import concourse.mybir as mybir
import concourse.tile as tile
from contextlib import ExitStack

f32 = mybir.dt.float32
bf16 = mybir.dt.bfloat16
i32 = mybir.dt.int32
AF = mybir.ActivationFunctionType
ALU = mybir.AluOpType
AX = mybir.AxisListType

# problem constants (hardcoded per spec)
B, N, MAX_T, PD = 128, 1024, 64, 85
NCLS = 79              # logits are pred[:, 6:85]
NC80 = 80              # padded class width
NCORES = 8
S = B // NCORES        # 16 samples per core
P = 128                # partitions
RCH = N // P           # 8 preds per partition per sample
J = MAX_T

ROWS = S * RCH         # 128 pred rows per partition, (s, r) order
SH = S // 2            # pair-phase half
GRIDH = [P, RCH, J, SH]
POOL_MASK = False       # gpsimd masks invalid-target x planes
POOL_A12 = False        # gpsimd computes a12 = pa + ta
POOL_GATHER = False     # gpsimd gathers field q=3 end-to-end
POOL_PM = False         # gpsimd computes CE pick multiply


def build_kernel(nc):
    preds_d = nc.dram_tensor("preds", [S, N, PD], f32, kind="ExternalInput")
    tgts_d = nc.dram_tensor("tgts", [S, MAX_T, 5], f32, kind="ExternalInput")
    loss_d = nc.dram_tensor("loss", [1, S], f32, kind="ExternalOutput")

    lp = nc.allow_low_precision("bf16 pipeline validated numerically vs reference (sim rel ~1.5e-3)")
    lp.__enter__()

    with tile.TileContext(nc) as tc, ExitStack() as ctx:
        sb = ctx.enter_context(tc.tile_pool(name="sb", bufs=1))

        # ---------- constants ----------
        ones_col = sb.tile([1, P], f32, tag="ones_col")       # lhsT (K=1, M=128)
        nc.vector.memset(ones_col[:], 1.0)
        ones_colb = sb.tile([1, P], bf16, tag="ones_colb")    # bf16 lhsT
        nc.vector.memset(ones_colb[:], 1.0)
        ONESB = sb.tile([P, 1], bf16, tag="onesb")            # lhsT for column sums
        nc.vector.memset(ONESB[:], 1.0)
        iot80_i = sb.tile([P, NC80], i32, tag="iot80_i")
        nc.gpsimd.iota(iot80_i[:], pattern=[[1, NC80]], base=0, channel_multiplier=0)
        IOTA80 = sb.tile([P, NC80], bf16, tag="iota80")
        nc.vector.tensor_copy(IOTA80[:], iot80_i[:])
        idn_i = sb.tile([P, P], i32, tag="idn_i")
        nc.gpsimd.iota(idn_i[:], pattern=[[1, P]], base=0, channel_multiplier=-1)
        IDENT = sb.tile([P, P], f32, tag="ident")
        nc.vector.tensor_scalar(IDENT[:], idn_i[:], 0, None, op0=ALU.is_equal)
        IDENTB = sb.tile([P, P], bf16, tag="identb")
        nc.vector.tensor_copy(IDENTB[:], IDENT[:])

        # ---------- persistent tiles ----------
        PRED = sb.tile([P, S, RCH, PD], f32, tag="pred")      # 43.5 KB
        BT5T = sb.tile([P, 5, J, S], bf16, tag="bt5t")        # targets, (j, s) layout
        PB = sb.tile([P, 5, RCH, S], bf16, tag="pb")          # pred x1,y1,x2,y2,area (r, s)
        IOUF = sb.tile([P, RCH, J, S], bf16, tag="iouf")      # 16 KB
        KV16 = sb.tile([1, S], f32, tag="kv16")               # valid-target count / sample
        BEST = sb.tile([P, RCH, S], bf16, tag="best")
        MR = sb.tile([P, RCH, S], bf16, tag="mr")
        MTC = sb.tile([P, RCH, S], bf16, tag="mtc")           # gathered cls
        MT4 = sb.tile([P, 4, RCH, S], bf16, tag="mt4")        # gathered box
        LBLT = sb.tile([P, S, RCH], bf16, tag="lblt")         # clipped label, (s, r)
        TAB = sb.tile([P, J, S], bf16, tag="tab")             # target areas
        L1S = sb.tile([P, ROWS], bf16, tag="l1s")             # ln(1+e^-|x|)
        MX0 = sb.tile([P, ROWS], bf16, tag="mx0")             # max(x,0)
        AXB = sb.tile([P, ROWS], bf16, tag="axbp")            # |x|
        SPP = sb.tile([P, S, RCH], bf16, tag="spp")
        SPN = sb.tile([P, S, RCH], bf16, tag="spn")

        # ---------- loads + target prep + pred extraction ----------
        with tc.tile_pool(name="ldp", bufs=1) as ldp:
            TROW = ldp.tile([1, S, MAX_T, 5], f32, tag="trow")
            nc.sync.dma_start(TROW[:], tgts_d[:].rearrange("s t c -> (s t c)").unsqueeze(0))
            for s4 in range(4):
                sl4 = slice(s4 * 4, (s4 + 1) * 4)
                src = preds_d[sl4].rearrange("s (p r) q -> p s r q", p=P)
                nc.sync.dma_start(PRED[:, sl4], src)

            # transpose all 5 target planes to (q, j, s) bf16 in one strided cast;
            # mask x planes (x1=x2=0 for invalid), area as 6th plane
            TROWT = ldp.tile([1, 6, J, S], bf16, tag="trowt")
            nc.vector.tensor_copy(TROWT[0:1, 0:5], TROW[:].rearrange("o s j q -> o q j s"))
            VB0T = ldp.tile([1, J, S], bf16, tag="vb0t")
            nc.vector.tensor_scalar(VB0T[:].rearrange("o j s -> o (j s)"),
                                    TROWT[0:1, 4].rearrange("o j s -> o (j s)"),
                                    0.0, None, op0=ALU.is_ge)
            nc.vector.tensor_tensor(TROWT[0:1, 0], TROWT[0:1, 0], VB0T[:], op=ALU.mult)
            nc.vector.tensor_tensor(TROWT[0:1, 2], TROWT[0:1, 2], VB0T[:], op=ALU.mult)
            TWH = ldp.tile([1, 2, J, S], bf16, tag="twh")
            nc.vector.tensor_tensor(TWH[:], TROWT[0:1, 2:4], TROWT[0:1, 0:2], op=ALU.subtract)
            nc.vector.tensor_tensor(TROWT[0:1, 5], TWH[0:1, 0], TWH[0:1, 1], op=ALU.mult)
            # kv count per sample: strided reduce over j
            nc.vector.tensor_reduce(KV16[:], VB0T[:].rearrange("o j s -> o s j"), axis=AX.X, op=ALU.add)

            # broadcast to all partitions via ones-matmuls, batched PSUM
            with tc.tile_pool(name="psa", bufs=1, space="PSUM") as psa:
                btA = psa.tile([P, 4 * J * S], f32, tag="bta")   # 16 KB: planes 0-3
                trf = TROWT[:].rearrange("o q j s -> o (q j s)")
                for h in range(8):
                    nc.tensor.matmul(btA[:, h * 512:(h + 1) * 512],
                                     ones_colb[:], trf[:, h * 512:(h + 1) * 512],
                                     start=True, stop=True)
                btv = btA[:].rearrange("p (q j s) -> p q j s", q=4, j=J)
                nc.scalar.copy(BT5T[:, 0:2], btv[:, 0:2])
                nc.scalar.copy(BT5T[:, 2:4], btv[:, 2:4])

            with tc.tile_pool(name="psb", bufs=1, space="PSUM") as psb:
                btB = psb.tile([P, 2 * J * S], f32, tag="btb")   # planes 4 (cls), 5 (area)
                for h in range(4):
                    nc.tensor.matmul(btB[:, h * 512:(h + 1) * 512],
                                     ones_colb[:], trf[:, 4096 + h * 512:4096 + (h + 1) * 512],
                                     start=True, stop=True)
                nc.scalar.copy(BT5T[:, 4], btB[:, 0:J * S].rearrange("p (j s) -> p j s", j=J))
                nc.scalar.copy(TAB[:], btB[:, J * S:2 * J * S].rearrange("p (j s) -> p j s", j=J))

            # pred fields transposed to (r, s), per DMA chunk
            for s4 in range(4):
                sl4 = slice(s4 * 4, (s4 + 1) * 4)
                for q in range(4):
                    nc.scalar.copy(PB[:, q, :, sl4], PRED[:, sl4, :, q].rearrange("p s r -> p r s"))
            # conf softplus pieces on Scalar: sp(x) = ln(1+e^-|x|) + max(x,0)
            CFV = PRED[:, :, :, 4].rearrange("p s r -> p (s r)")
            AXC = ldp.tile([P, ROWS], f32, tag="axc")
            nc.scalar.activation(AXC[:], CFV, AF.Abs)
            EN = ldp.tile([P, ROWS], f32, tag="en")
            nc.scalar.activation(EN[:], AXC[:], AF.Exp, scale=-1.0)
            nc.scalar.activation(L1S[:], EN[:], AF.Ln, bias=1.0)
            nc.scalar.activation(MX0[:], CFV, AF.Relu)
            nc.scalar.copy(AXB[:], AXC[:])

        # ---------- pair phase in s-halves ----------
        with tc.tile_pool(name="pp", bufs=1) as pp:
            aeng = nc.gpsimd if POOL_A12 else nc.vector
            SQ = S // 4
            for h in range(2):
                ssl = slice(h * SH, (h + 1) * SH)

                def pbch(q):
                    return PB[:, q, :, ssl].unsqueeze(2).broadcast_to(GRIDH)

                def tbch(q):
                    return BT5T[:, q, :, ssl].unsqueeze(1).broadcast_to(GRIDH)

                GA = pp.tile(GRIDH, bf16, tag="ga")
                GB = pp.tile(GRIDH, bf16, tag="gb")
                GC = pp.tile(GRIDH, bf16, tag="gc")
                GD = pp.tile(GRIDH, bf16, tag="gd")
                # pred areas for this half
                PWH = pp.tile([P, RCH, SH], bf16, tag="pwh")
                nc.vector.tensor_tensor(PWH[:], PB[:, 2, :, ssl], PB[:, 0, :, ssl], op=ALU.subtract)
                PHH = pp.tile([P, RCH, SH], bf16, tag="phh")
                nc.vector.tensor_tensor(PHH[:], PB[:, 3, :, ssl], PB[:, 1, :, ssl], op=ALU.subtract)
                nc.vector.tensor_tensor(PB[:, 4, :, ssl], PWH[:], PHH[:], op=ALU.mult)
                nc.vector.tensor_tensor(GA[:], pbch(0), tbch(0), op=ALU.max)     # ix1
                nc.vector.tensor_tensor(GB[:], pbch(2), tbch(2), op=ALU.min)     # ix2
                nc.vector.tensor_tensor(GC[:], GB[:], GA[:], op=ALU.subtract)    # wx
                gcf = GC[:].rearrange("p r j s -> p (r j s)")
                nc.vector.tensor_scalar(gcf, gcf, 0.0, None, op0=ALU.max)        # relu
                nc.vector.tensor_tensor(GD[:], pbch(1), tbch(1), op=ALU.max)     # iy1
                nc.vector.tensor_tensor(GA[:], pbch(3), tbch(3), op=ALU.min)     # iy2
                nc.vector.tensor_tensor(GB[:], GA[:], GD[:], op=ALU.subtract)    # wy
                gbf = GB[:].rearrange("p r j s -> p (r j s)")
                nc.vector.tensor_scalar(gbf, gbf, 0.0, None, op0=ALU.max)        # relu
                nc.vector.tensor_tensor(GD[:], GC[:], GB[:], op=ALU.mult)        # inter
                A12 = pp.tile(GRIDH, bf16, tag="a12", bufs=2)
                tabb = TAB[:, :, ssl].unsqueeze(1).broadcast_to(GRIDH)
                aeng.tensor_tensor(A12[:], pbch(4), tabb, op=ALU.add)            # pa + ta
                nc.vector.tensor_tensor(GC[:], A12[:], GD[:], op=ALU.subtract)   # den bf16

                # reciprocal quarters: Scalar upcast(+1e-6) -> DVE rcp -> Scalar downcast
                for hq in range(2):
                    qsl = slice(hq * SQ, (hq + 1) * SQ)
                    osl = slice(h * SH + hq * SQ, h * SH + (hq + 1) * SQ)
                    DENF = pp.tile([P, RCH, J, SQ], f32, tag="denf", bufs=2)
                    nc.scalar.activation(DENF[:], GC[:, :, :, qsl], AF.Copy, bias=1e-6)
                    RCPF = pp.tile([P, RCH, J, SQ], f32, tag="rcpf", bufs=2)
                    nc.vector.reciprocal_approx_fast(
                        RCPF[:].rearrange("p r j s -> p (r j s)"),
                        DENF[:].rearrange("p r j s -> p (r j s)"))
                    RCPB = pp.tile([P, RCH, J, SQ], bf16, tag="rcpb", bufs=2)
                    nc.scalar.copy(RCPB[:], RCPF[:])
                    nc.vector.tensor_tensor(IOUF[:, :, :, osl], GD[:, :, :, qsl], RCPB[:], op=ALU.mult)

                # BEST via tree-max over j for this half (fills rcp-wait gaps)
                cur = IOUF[:, :, :, ssl]
                width = J
                while width > 2:
                    half = width // 2
                    nt = pp.tile([P, RCH, half, SH], bf16, tag=f"bmh{half}")
                    nc.vector.tensor_tensor(nt[:], cur[:, :, 0:half], cur[:, :, half:width], op=ALU.max)
                    cur = nt[:]
                    width = half
                nc.vector.tensor_tensor(BEST[:, :, ssl], cur[:, :, 0], cur[:, :, 1], op=ALU.max)

        # conf softplus assembly
        MXN = sb.tile([P, ROWS], bf16, tag="mxn")   # max(-x,0) = |x| - max(x,0)
        nc.vector.tensor_tensor(MXN[:], AXB[:], MX0[:], op=ALU.subtract)
        nc.vector.tensor_tensor(SPP[:].rearrange("p s r -> p (s r)"), L1S[:], MX0[:], op=ALU.add)
        nc.vector.tensor_tensor(SPN[:].rearrange("p s r -> p (s r)"), L1S[:], MXN[:], op=ALU.add)

        nc.vector.tensor_scalar(MR[:].rearrange("p r s -> p (r s)"),
                                BEST[:].rearrange("p r s -> p (r s)"), 0.5, None, op0=ALU.is_gt)

        # ---------- fallback-mask chain (PE/Scalar parts overlap gather) ----------
        pst = ctx.enter_context(tc.tile_pool(name="pst", bufs=1, space="PSUM"))
        B4 = sb.tile([P, 4, S], bf16, tag="b4")
        nc.vector.tensor_tensor(B4[:], BEST[:, 0:4], BEST[:, 4:8], op=ALU.max)
        B2 = sb.tile([P, 2, S], bf16, tag="b2")
        nc.vector.tensor_tensor(B2[:], B4[:, 0:2], B4[:, 2:4], op=ALU.max)
        BESTS16 = sb.tile([P, S], bf16, tag="bests16")
        nc.vector.tensor_tensor(BESTS16[:], B2[:, 0], B2[:, 1], op=ALU.max)
        trb = pst.tile([S, P], bf16, tag="tp128")
        nc.tensor.transpose(trb[:], BESTS16[:], IDENTB[:])
        TB = sb.tile([S, P], f32, tag="tb")
        nc.scalar.copy(TB[:], trb[:])
        GMAX16 = sb.tile([S, 1], f32, tag="gmax16")
        nc.vector.tensor_reduce(GMAX16[:], TB[:], axis=AX.X, op=ALU.max)
        EQT = sb.tile([S, P], f32, tag="eqt")
        nc.vector.tensor_scalar(EQT[:], TB[:], GMAX16[:], None, op0=ALU.is_equal)
        NAFT = sb.tile([S, 1], f32, tag="naft")
        nc.vector.tensor_scalar(NAFT[:], GMAX16[:], 0.5, None, op0=ALU.is_le)
        NF128 = sb.tile([S, P], f32, tag="nf128")
        nc.vector.tensor_scalar(NF128[:], TB[:], 0.0, NAFT[:], op0=ALU.mult, op1=ALU.add)
        teqc = pst.tile([P, S], f32, tag="tpb")
        nc.tensor.transpose(teqc[:], EQT[:], IDENT[:S, :S])
        EQC = sb.tile([P, S], bf16, tag="eqc")
        nc.scalar.copy(EQC[:], teqc[:])
        tnaf = pst.tile([P, S], f32, tag="tpc")
        nc.tensor.transpose(tnaf[:], NF128[:], IDENT[:S, :S])
        NAFC = sb.tile([P, S], bf16, tag="nafc")
        nc.scalar.copy(NAFC[:], tnaf[:])
        ECN = sb.tile([P, S], bf16, tag="ecn")
        nc.vector.tensor_tensor(ECN[:], EQC[:], NAFC[:], op=ALU.mult)

        # ---------- eq + gather (cls first so CE label prep starts early) ----------
        GRID = [P, RCH, J, S]

        def tbc(q):
            return BT5T[:, q].unsqueeze(1).broadcast_to(GRID)

        with tc.tile_pool(name="tg", bufs=1) as tg:
            EQ = tg.tile(GRID, bf16, tag="eq")
            best_b = BEST[:].unsqueeze(2).broadcast_to(GRID)
            nc.vector.tensor_tensor(EQ[:], IOUF[:], best_b, op=ALU.is_equal)

            GT8 = tg.tile([P, 5, RCH, 8, S], bf16, tag="gt8")
            # Pool gathers q=3 end-to-end
            if POOL_GATHER:
                gpp = tg.tile(GRID, bf16, tag="gpp")
                nc.gpsimd.tensor_tensor(gpp[:], EQ[:], tbc(3), op=ALU.mult)
                g32p = tg.tile([P, RCH, 32, S], bf16, tag="g32p")
                nc.gpsimd.tensor_tensor(g32p[:], gpp[:, :, 0:32], gpp[:, :, 32:64], op=ALU.add)
                g16p = tg.tile([P, RCH, 16, S], bf16, tag="g16p")
                nc.gpsimd.tensor_tensor(g16p[:], g32p[:, :, 0:16], g32p[:, :, 16:32], op=ALU.add)
                nc.gpsimd.tensor_tensor(GT8[:, 3], g16p[:, :, 0:8], g16p[:, :, 8:16], op=ALU.add)

            qlist = [4, 0, 1] + ([] if POOL_GATHER else [3]) + [2]
            for q in qlist:
                gp = tg.tile(GRID, bf16, tag="gp", bufs=1)
                nc.vector.tensor_tensor(gp[:], EQ[:], tbc(q), op=ALU.mult)
                g32 = tg.tile([P, RCH, 32, S], bf16, tag="g32")
                nc.vector.tensor_tensor(g32[:], gp[:, :, 0:32], gp[:, :, 32:64], op=ALU.add)
                g16 = tg.tile([P, RCH, 16, S], bf16, tag="g16")
                nc.vector.tensor_tensor(g16[:], g32[:, :, 0:16], g32[:, :, 16:32], op=ALU.add)
                nc.vector.tensor_tensor(GT8[:, q], g16[:, :, 0:8], g16[:, :, 8:16], op=ALU.add)
                if q == 4:
                    # finish cls tail immediately: 8 -> 1, then label prep
                    c4 = tg.tile([P, RCH, 4, S], bf16, tag="c4")
                    nc.vector.tensor_tensor(c4[:], GT8[:, 4, :, 0:4], GT8[:, 4, :, 4:8], op=ALU.add)
                    c2 = tg.tile([P, RCH, 2, S], bf16, tag="c2")
                    nc.vector.tensor_tensor(c2[:], c4[:, :, 0:2], c4[:, :, 2:4], op=ALU.add)
                    nc.vector.tensor_tensor(MTC[:], c2[:, :, 0], c2[:, :, 1], op=ALU.add)
                    LBLC = tg.tile([P, RCH, S], bf16, tag="lblc")
                    nc.vector.tensor_scalar(LBLC[:].rearrange("p r s -> p (r s)"),
                                            MTC[:].rearrange("p r s -> p (r s)"),
                                            0.0, float(NCLS - 1), op0=ALU.max, op1=ALU.min)
                    nc.scalar.copy(LBLT[:], LBLC[:].rearrange("p r s -> p s r"))

            # shared box tail: 8 -> 4 -> 2 -> 1 over the 4 coord fields
            bt4 = GT8[:, 0:4].rearrange("p q r j s -> p (q r) j s")
            t84 = tg.tile([P, 4 * RCH, 4, S], bf16, tag="t84")
            nc.vector.tensor_tensor(t84[:], bt4[:, :, 0:4], bt4[:, :, 4:8], op=ALU.add)
            t42 = tg.tile([P, 4 * RCH, 2, S], bf16, tag="t42")
            nc.vector.tensor_tensor(t42[:], t84[:, :, 0:2], t84[:, :, 2:4], op=ALU.add)
            nc.vector.tensor_tensor(MT4[:].rearrange("p q r s -> p (q r) s"),
                                    t42[:, :, 0], t42[:, :, 1], op=ALU.add)

            # ---------- CE (4 chunks, Scalar prefetch overlaps gather DVE work) ----------
            HR = ROWS // 4
            SP2 = sb.tile([P, 2, ROWS], f32, tag="sp2")   # [:,0]=sumexp, [:,1]=pick
            with tc.tile_pool(name="cp", bufs=1) as cp:
                for ch in range(4):
                    rs = slice(ch * (S // 4), (ch + 1) * (S // 4))
                    fs = slice(ch * HR, (ch + 1) * HR)
                    LBLR = cp.tile([P, HR, NC80], bf16, tag="lblr", bufs=2)
                    nc.scalar.copy(LBLR[:], LBLT[:].rearrange("p s r -> p (s r)")[:, fs]
                                   .unsqueeze(2).broadcast_to([P, HR, NC80]))
                    EP = cp.tile([P, 2, HR, NC80], bf16, tag="ep", bufs=2)  # [e2 | pick]
                    nc.vector.memset(EP[:, 0, :, NCLS:NC80], 0.0)
                    logits = PRED[:, rs, :, 6:].rearrange("p s r c -> p (s r) c")
                    nc.scalar.activation(EP[:, 0, :, 0:NCLS], logits, AF.Exp)
                    ohc = cp.tile([P, HR, NC80], bf16, tag="ohc", bufs=2)
                    iot_b = IOTA80[:].unsqueeze(1).broadcast_to([P, HR, NC80])
                    nc.vector.tensor_tensor(ohc[:], iot_b, LBLR[:], op=ALU.is_equal)
                    nc.vector.tensor_tensor(EP[:, 1], ohc[:], EP[:, 0], op=ALU.mult)
                    # joint tree: 80 -> 40 -> 20 -> 10 -> 5 -> reduce
                    cur = EP[:]
                    width = NC80
                    while width > 5:
                        half = width // 2
                        nt = cp.tile([P, 2, HR, half], bf16, tag=f"se{half}", name="nt")
                        nc.vector.tensor_tensor(nt[:], cur[:, :, :, 0:half], cur[:, :, :, half:width], op=ALU.add)
                        cur = nt[:]
                        width = half
                    nc.vector.tensor_reduce(SP2[:, :, fs], cur, axis=AX.X, op=ALU.add)


        # ---------- smooth L1 ((r, s) layout; x2 folded into transpose scale) ----------
        SL1T = sb.tile([P, S, RCH], bf16, tag="sl1t")
        with tc.tile_pool(name="sp", bufs=1) as sp:
            d = sp.tile([P, 4, RCH, S], bf16, tag="d")
            nc.vector.tensor_tensor(d[:], PB[:, 0:4], MT4[:], op=ALU.subtract)
            ad = sp.tile([P, 4, RCH, S], bf16, tag="ad")
            nc.scalar.activation(ad[:], d[:], AF.Abs)                        # |d|
            tmh = sp.tile([P, 4, RCH, S], bf16, tag="tmh")
            nc.vector.tensor_scalar(tmh[:].rearrange("p q r s -> p (q r s)"),
                                    ad[:].rearrange("p q r s -> p (q r s)"),
                                    1.0, 0.5, op0=ALU.min, op1=ALU.mult)     # min(|d|,1)/2
            uu = sp.tile([P, 4, RCH, S], bf16, tag="uu")
            nc.vector.tensor_tensor(uu[:], ad[:], tmh[:], op=ALU.subtract)   # |d| - tm/2
            sl1h = sp.tile([P, 4, RCH, S], bf16, tag="sl1h")
            nc.vector.tensor_tensor(sl1h[:], tmh[:], uu[:], op=ALU.mult)     # sl1/2
            q2 = sp.tile([P, 2, RCH, S], bf16, tag="q2")
            nc.vector.tensor_tensor(q2[:], sl1h[:, 0:2], sl1h[:, 2:4], op=ALU.add)
            shs = sp.tile([P, RCH, S], bf16, tag="shs")
            nc.vector.tensor_tensor(shs[:], q2[:, 0], q2[:, 1], op=ALU.add)
            nc.scalar.activation(SL1T[:], shs[:].rearrange("p r s -> p s r"),
                                 AF.Copy, scale=2.0)                          # x2 fold

        # ---------- final match mask + weighted sums ----------
        EQB = sb.tile([P, RCH, S], bf16, tag="eqb")
        nc.vector.tensor_tensor(EQB[:], BEST[:],
                                BESTS16[:].unsqueeze(1).broadcast_to([P, RCH, S]), op=ALU.is_equal)
        M2 = sb.tile([P, RCH, S], bf16, tag="m2")
        nc.vector.tensor_tensor(M2[:], EQB[:],
                                ECN[:].unsqueeze(1).broadcast_to([P, RCH, S]), op=ALU.mult)
        MM = sb.tile([P, RCH, S], bf16, tag="mm")
        nc.vector.tensor_tensor(MM[:], MR[:], M2[:], op=ALU.add)

        FQ = sb.tile([P, 6, S, RCH], bf16, tag="fq")
        nc.scalar.copy(FQ[:, 0], MM[:].rearrange("p r s -> p s r"))
        nc.vector.tensor_tensor(FQ[:, 1], FQ[:, 0], SL1T[:], op=ALU.mult)
        nc.vector.tensor_tensor(FQ[:, 3], FQ[:, 0], SPN[:], op=ALU.mult)
        nc.vector.tensor_tensor(FQ[:, 4], FQ[:, 0], SPP[:], op=ALU.mult)
        nc.vector.tensor_copy(FQ[:, 5], SPP[:])

        # ---------- partition sums via ones-matmul (q=2 deferred until CE) ----------
        R768 = sb.tile([1, 6, S, RCH], f32, tag="r768")
        fqf = FQ[:].rearrange("p q s r -> p (q s r)")
        psr = ctx.enter_context(tc.tile_pool(name="psr", bufs=1, space="PSUM"))
        for lo, hi in ((0, 256), (384, 768)):
            rq_ps = psr.tile([1, 384], f32, tag="rq_ps", bufs=3, name=f"rq{lo}")
            nc.tensor.matmul(rq_ps[:, 0:hi - lo], ONESB[:], fqf[:, lo:hi], start=True, stop=True)
            nc.vector.tensor_copy(R768[:].rearrange("o q s r -> o (q s r)")[:, lo:hi], rq_ps[:, 0:hi - lo])

        # per-sample scalars that don't depend on CE: compute before CE finishes
        RQ = sb.tile([1, 6, S], f32, tag="rq")
        for q in (0, 1, 3, 4, 5):
            nc.vector.tensor_reduce(RQ[:, q], R768[:, q], axis=AX.X, op=ALU.add)
        mcnt = RQ[:, 0]; bbox_n = RQ[:, 1]
        spn_n = RQ[:, 3]; spp_m = RQ[:, 4]; spp_all = RQ[:, 5]

        def t16(tag):
            return sb.tile([1, S], f32, tag=tag, name=tag)

        d4 = t16("d4"); nc.vector.tensor_scalar(d4[:], mcnt, 4.0, 1.0, op0=ALU.mult, op1=ALU.max)
        r4 = t16("r4"); nc.vector.reciprocal(r4[:], d4[:])
        bbox = t16("bbox"); nc.vector.tensor_tensor(bbox[:], bbox_n, r4[:], op=ALU.mult)
        d1 = t16("d1"); nc.vector.tensor_scalar(d1[:], mcnt, 1.0, None, op0=ALU.max)
        r1 = t16("r1"); nc.vector.reciprocal(r1[:], d1[:])
        confm = t16("confm"); nc.vector.tensor_tensor(confm[:], spn_n, r1[:], op=ALU.mult)
        ucnt = t16("ucnt"); nc.vector.tensor_scalar(ucnt[:], mcnt, -1.0, float(N), op0=ALU.mult, op1=ALU.add)
        du = t16("du"); nc.vector.tensor_scalar(du[:], ucnt[:], 1.0, None, op0=ALU.max)
        ru = t16("ru"); nc.vector.reciprocal(ru[:], du[:])
        cun = t16("cun"); nc.vector.tensor_tensor(cun[:], spp_all, spp_m, op=ALU.subtract)
        confu = t16("confu"); nc.vector.tensor_tensor(confu[:], cun[:], ru[:], op=ALU.mult)
        csum = t16("csum"); nc.vector.tensor_tensor(csum[:], confm[:], confu[:], op=ALU.add)
        chalf = t16("chalf"); nc.vector.tensor_scalar(chalf[:], csum[:], 0.5, None, op0=ALU.mult)
        ug = t16("ug"); nc.vector.tensor_scalar(ug[:], ucnt[:], 0.0, None, op0=ALU.is_gt)
        ugn = t16("ugn"); nc.vector.tensor_scalar(ugn[:], ucnt[:], 0.0, None, op0=ALU.is_le)
        c1 = t16("c1"); nc.vector.tensor_tensor(c1[:], chalf[:], ug[:], op=ALU.mult)
        c2 = t16("c2"); nc.vector.tensor_tensor(c2[:], confm[:], ugn[:], op=ALU.mult)
        confL = t16("confL"); nc.vector.tensor_tensor(confL[:], c1[:], c2[:], op=ALU.add)
        lnv = t16("lnv"); nc.vector.tensor_scalar(lnv[:], spp_all, 1.0 / float(N), None, op0=ALU.mult)
        kvg = t16("kvg"); nc.vector.tensor_scalar(kvg[:], KV16[:], 0.0, None, op0=ALU.is_gt)
        kvn = t16("kvn"); nc.vector.tensor_scalar(kvn[:], KV16[:], 0.0, None, op0=ALU.is_le)
        bc = t16("bc"); nc.vector.tensor_tensor(bc[:], bbox[:], confL[:], op=ALU.add)

        LL2 = sb.tile([P, 2, ROWS], f32, tag="ll2")
        nc.scalar.activation(LL2[:], SP2[:], AF.Ln)
        CET = sb.tile([P, S, RCH], bf16, tag="cet")
        nc.vector.tensor_tensor(CET[:], LL2[:, 0].rearrange("p (s r) -> p s r", s=S),
                                LL2[:, 1].rearrange("p (s r) -> p s r", s=S), op=ALU.subtract)

        nc.vector.tensor_tensor(FQ[:, 2], FQ[:, 0], CET[:], op=ALU.mult)
        rq_ps2 = psr.tile([1, 384], f32, tag="rq_ps", bufs=3, name="rq2")
        nc.tensor.matmul(rq_ps2[:, 0:128], ONESB[:], fqf[:, 256:384], start=True, stop=True)
        nc.vector.tensor_copy(R768[:].rearrange("o q s r -> o (q s r)")[:, 256:384], rq_ps2[:, 0:128])

        # ---------- final: CE-dependent tail ----------
        nc.vector.tensor_reduce(RQ[:, 2], R768[:, 2], axis=AX.X, op=ALU.add)
        cls_n = RQ[:, 2]
        clsl = t16("clsl"); nc.vector.tensor_tensor(clsl[:], cls_n, r1[:], op=ALU.mult)
        lv = t16("lv"); nc.vector.tensor_tensor(lv[:], bc[:], clsl[:], op=ALU.add)
        lA = t16("lA"); nc.vector.tensor_tensor(lA[:], lv[:], kvg[:], op=ALU.mult)
        lB = t16("lB"); nc.vector.tensor_tensor(lB[:], lnv[:], kvn[:], op=ALU.mult)
        LROW = t16("lrow"); nc.vector.tensor_tensor(LROW[:], lA[:], lB[:], op=ALU.add)
        nc.sync.dma_start(loss_d[:], LROW[:])

    lp.__exit__(None, None, None)
    return preds_d, tgts_d, loss_d


_NC_CACHE = {}


def get_nc():
    if "nc" not in _NC_CACHE:
        nc = bacc.Bacc("TRN2", target_bir_lowering=False, debug=False)
        build_kernel(nc)
        nc.compile()
        _NC_CACHE["nc"] = nc
    return _NC_CACHE["nc"]


def kernel(preds: np.ndarray, targets: np.ndarray) -> np.ndarray:
    from concourse.bass_utils import run_bass_kernel_spmd

    nc = get_nc()
    in_maps = []
    for c in range(NCORES):
        in_maps.append({
            "preds": np.ascontiguousarray(preds[c * S:(c + 1) * S], dtype=np.float32),
            "tgts": np.ascontiguousarray(targets[c * S:(c + 1) * S], dtype=np.float32),
        })
    res = run_bass_kernel_spmd(nc, in_maps, core_ids=list(range(NCORES)))
    per_sample = np.concatenate([res.results[c]["loss"].reshape(-1) for c in range(NCORES)])
    return np.float32(per_sample.sum() / B)


# revision 23
# speedup vs baseline: 1.0145x; 1.0145x over previous
"""Trainium2 Bass kernel for nn_DetectionLoss (B=128, N=1024, MAX_T=64, 80 classes).

Contract: kernel(**inputs) takes FULL inputs {preds: (128,1024,85) f32,
targets: (128,64,5) f32} and returns the FULL scalar output (f32 (),
mean of per-sample losses), computed data-parallel on 8 NeuronCores
(16 samples per core).

v4 design (vs v2 baseline at ~207us, v3 at ~211us):
- Grid layout [P, r, j, s] with s innermost: broadcasts are stride-0 MIDDLE
  dims, so every DVE tensor_tensor runs in 2x mode with zero replicate copies.
- Targets broadcast via 10 ones-matmuls into batched PSUM, evacuated with
  TWO large Scalar copies (v3 did 10 small ones, latency-serialized).
- Invalid targets: x1=x2=0 via vb-multiply on Pool (inter==0, never wins).
- Pair phase in 2 sample-halves; Pool computes a12 = pa+ta concurrently.
- Division: den bf16 (2x) -> Scalar upcast +1e-6 bias -> DVE
  reciprocal_approx_fast (only 1x op) -> Scalar downcast, 4-quarter pipeline.
- Gather: eq is_eq at 2x; field q=3 gathered fully on Pool, rest on DVE.
- CE: Scalar exp/label-broadcast lead DVE trees; pick multiply on Pool.
- Fallback-mask chain placed right after BEST so its PE transposes +
  Scalar copies overlap gather.
"""
import numpy as np

import concourse.bass as bass
import concourse.bacc as bacc
import concourse.mybir as mybir
import concourse.tile as tile
from contextlib import ExitStack

f32 = mybir.dt.float32
bf16 = mybir.dt.bfloat16
i32 = mybir.dt.int32
AF = mybir.ActivationFunctionType
ALU = mybir.AluOpType
AX = mybir.AxisListType

# problem constants (hardcoded per spec)
B, N, MAX_T, PD = 128, 1024, 64, 85
NCLS = 79              # logits are pred[:, 6:85]
NC80 = 80              # padded class width
NCORES = 8
S = B // NCORES        # 16 samples per core
P = 128                # partitions
RCH = N // P           # 8 preds per partition per sample
J = MAX_T

ROWS = S * RCH         # 128 pred rows per partition, (s, r) order
SH = S // 2            # pair-phase half
GRIDH = [P, RCH, J, SH]
POOL_MASK = False       # gpsimd masks invalid-target x planes
POOL_A12 = False        # gpsimd computes a12 = pa + ta
POOL_GATHER = False     # gpsimd gathers field q=3 end-to-end
POOL_PM = False         # gpsimd computes CE pick multiply


def build_kernel(nc):
    preds_d = nc.dram_tensor("preds", [S, N, PD], f32, kind="ExternalInput")
    tgts_d = nc.dram_tensor("tgts", [S, MAX_T, 5], f32, kind="ExternalInput")
    loss_d = nc.dram_tensor("loss", [1, S], f32, kind="ExternalOutput")

    lp = nc.allow_low_precision("bf16 pipeline validated numerically vs reference (sim rel ~1.5e-3)")
    lp.__enter__()

    with tile.TileContext(nc) as tc, ExitStack() as ctx:
        sb = ctx.enter_context(tc.tile_pool(name="sb", bufs=1))

        # ---------- constants ----------
        ones_col = sb.tile([1, P], f32, tag="ones_col")       # lhsT (K=1, M=128)
        nc.vector.memset(ones_col[:], 1.0)
        ones_colb = sb.tile([1, P], bf16, tag="ones_colb")    # bf16 lhsT
        nc.vector.memset(ones_colb[:], 1.0)
        ONESB = sb.tile([P, 1], bf16, tag="onesb")            # lhsT for column sums
        nc.vector.memset(ONESB[:], 1.0)
        iot80_i = sb.tile([P, NC80], i32, tag="iot80_i")
        nc.gpsimd.iota(iot80_i[:], pattern=[[1, NC80]], base=0, channel_multiplier=0)
        IOTA80 = sb.tile([P, NC80], bf16, tag="iota80")
        nc.vector.tensor_copy(IOTA80[:], iot80_i[:])
        idn_i = sb.tile([P, P], i32, tag="idn_i")
        nc.gpsimd.iota(idn_i[:], pattern=[[1, P]], base=0, channel_multiplier=-1)
        IDENT = sb.tile([P, P], f32, tag="ident")
        nc.vector.tensor_scalar(IDENT[:], idn_i[:], 0, None, op0=ALU.is_equal)
        IDENTB = sb.tile([P, P], bf16, tag="identb")
        nc.vector.tensor_copy(IDENTB[:], IDENT[:])

        # ---------- persistent tiles ----------
        PRED = sb.tile([P, S, RCH, PD], f32, tag="pred")      # 43.5 KB
        BT5T = sb.tile([P, 5, J, S], bf16, tag="bt5t")        # targets, (j, s) layout
        PB = sb.tile([P, 5, RCH, S], bf16, tag="pb")          # pred x1,y1,x2,y2,area (r, s)
        IOUF = sb.tile([P, RCH, J, S], bf16, tag="iouf")      # 16 KB
        KV16 = sb.tile([1, S], f32, tag="kv16")               # valid-target count / sample
        BEST = sb.tile([P, RCH, S], bf16, tag="best")
        MR = sb.tile([P, RCH, S], bf16, tag="mr")
        MTC = sb.tile([P, RCH, S], bf16, tag="mtc")           # gathered cls
        MT4 = sb.tile([P, 4, RCH, S], bf16, tag="mt4")        # gathered box
        LBLT = sb.tile([P, S, RCH], bf16, tag="lblt")         # clipped label, (s, r)
        TAB = sb.tile([P, J, S], bf16, tag="tab")             # target areas
        L1S = sb.tile([P, ROWS], bf16, tag="l1s")             # ln(1+e^-|x|)
        MX0 = sb.tile([P, ROWS], bf16, tag="mx0")             # max(x,0)
        AXB = sb.tile([P, ROWS], bf16, tag="axbp")            # |x|
        SPP = sb.tile([P, S, RCH], bf16, tag="spp")
        SPN = sb.tile([P, S, RCH], bf16, tag="spn")

        # ---------- loads + target prep + pred extraction ----------
        with tc.tile_pool(name="ldp", bufs=1) as ldp:
            TROW = ldp.tile([1, S, MAX_T, 5], f32, tag="trow")
            nc.sync.dma_start(TROW[:], tgts_d[:].rearrange("s t c -> (s t c)").unsqueeze(0))
            for s4 in range(4):
                sl4 = slice(s4 * 4, (s4 + 1) * 4)
                src = preds_d[sl4].rearrange("s (p r) q -> p s r q", p=P)
                nc.sync.dma_start(PRED[:, sl4], src)

            # transpose target planes to (q, j, s) bf16: Scalar does 4,1,3 while
            # DVE does 0,2 (+ masking), minimizing the critical path to the matmuls
            TROWT = ldp.tile([1, 6, J, S], bf16, tag="trowt")
            nc.scalar.copy(TROWT[0:1, 4], TROW[0:1, :, :, 4].rearrange("o s j -> o j s"))
            nc.scalar.copy(TROWT[0:1, 1], TROW[0:1, :, :, 1].rearrange("o s j -> o j s"))
            nc.scalar.copy(TROWT[0:1, 3], TROW[0:1, :, :, 3].rearrange("o s j -> o j s"))
            VB0T = ldp.tile([1, J, S], bf16, tag="vb0t")
            nc.vector.tensor_scalar(VB0T[:].rearrange("o j s -> o (j s)"),
                                    TROWT[0:1, 4].rearrange("o j s -> o (j s)"),
                                    0.0, None, op0=ALU.is_ge)
            nc.vector.tensor_tensor(TROWT[0:1, 0], TROW[0:1, :, :, 0].rearrange("o s j -> o j s"),
                                    VB0T[:], op=ALU.mult)
            nc.vector.tensor_tensor(TROWT[0:1, 2], TROW[0:1, :, :, 2].rearrange("o s j -> o j s"),
                                    VB0T[:], op=ALU.mult)
            TWH = ldp.tile([1, 2, J, S], bf16, tag="twh")
            nc.vector.tensor_tensor(TWH[:], TROWT[0:1, 2:4], TROWT[0:1, 0:2], op=ALU.subtract)
            nc.vector.tensor_tensor(TROWT[0:1, 5], TWH[0:1, 0], TWH[0:1, 1], op=ALU.mult)
            # kv count per sample: strided reduce over j
            nc.vector.tensor_reduce(KV16[:], VB0T[:].rearrange("o j s -> o s j"), axis=AX.X, op=ALU.add)

            # pred fields transposed to (r, s), per DMA chunk
            for s4 in range(4):
                sl4 = slice(s4 * 4, (s4 + 1) * 4)
                for q in range(4):
                    nc.scalar.copy(PB[:, q, :, sl4], PRED[:, sl4, :, q].rearrange("p s r -> p r s"))

            # broadcast to all partitions via ones-matmuls, batched PSUM
            with tc.tile_pool(name="psa", bufs=1, space="PSUM") as psa:
                btA = psa.tile([P, 4 * J * S], f32, tag="bta")   # 16 KB: planes 0-3
                trf = TROWT[:].rearrange("o q j s -> o (q j s)")
                for h in range(8):
                    nc.tensor.matmul(btA[:, h * 512:(h + 1) * 512],
                                     ones_colb[:], trf[:, h * 512:(h + 1) * 512],
                                     start=True, stop=True)
                btv = btA[:].rearrange("p (q j s) -> p q j s", q=4, j=J)
                nc.scalar.copy(BT5T[:, 0:2], btv[:, 0:2])
                nc.scalar.copy(BT5T[:, 2:4], btv[:, 2:4])

            with tc.tile_pool(name="psb", bufs=1, space="PSUM") as psb:
                btB = psb.tile([P, 2 * J * S], f32, tag="btb")   # planes 4 (cls), 5 (area)
                for h in range(4):
                    nc.tensor.matmul(btB[:, h * 512:(h + 1) * 512],
                                     ones_colb[:], trf[:, 4096 + h * 512:4096 + (h + 1) * 512],
                                     start=True, stop=True)
                nc.scalar.copy(BT5T[:, 4], btB[:, 0:J * S].rearrange("p (j s) -> p j s", j=J))
                nc.scalar.copy(TAB[:], btB[:, J * S:2 * J * S].rearrange("p (j s) -> p j s", j=J))

            # conf softplus pieces on Scalar: sp(x) = ln(1+e^-|x|) + max(x,0)
            CFV = PRED[:, :, :, 4].rearrange("p s r -> p (s r)")
            AXC = ldp.tile([P, ROWS], f32, tag="axc")
            nc.scalar.activation(AXC[:], CFV, AF.Abs)
            EN = ldp.tile([P, ROWS], f32, tag="en")
            nc.scalar.activation(EN[:], AXC[:], AF.Exp, scale=-1.0)
            nc.scalar.activation(L1S[:], EN[:], AF.Ln, bias=1.0)
            nc.scalar.activation(MX0[:], CFV, AF.Relu)
            nc.scalar.copy(AXB[:], AXC[:])

        # ---------- pair phase in s-halves ----------
        with tc.tile_pool(name="pp", bufs=1) as pp:
            aeng = nc.gpsimd if POOL_A12 else nc.vector
            SQ = S // 4
            for h in range(2):
                ssl = slice(h * SH, (h + 1) * SH)

                def pbch(q):
                    return PB[:, q, :, ssl].unsqueeze(2).broadcast_to(GRIDH)

                def tbch(q):
                    return BT5T[:, q, :, ssl].unsqueeze(1).broadcast_to(GRIDH)

                GA = pp.tile(GRIDH, bf16, tag="ga")
                GB = pp.tile(GRIDH, bf16, tag="gb")
                GC = pp.tile(GRIDH, bf16, tag="gc")
                GD = pp.tile(GRIDH, bf16, tag="gd")
                # pred areas for this half
                PWH = pp.tile([P, RCH, SH], bf16, tag="pwh")
                nc.vector.tensor_tensor(PWH[:], PB[:, 2, :, ssl], PB[:, 0, :, ssl], op=ALU.subtract)
                PHH = pp.tile([P, RCH, SH], bf16, tag="phh")
                nc.vector.tensor_tensor(PHH[:], PB[:, 3, :, ssl], PB[:, 1, :, ssl], op=ALU.subtract)
                nc.vector.tensor_tensor(PB[:, 4, :, ssl], PWH[:], PHH[:], op=ALU.mult)
                nc.vector.tensor_tensor(GA[:], pbch(0), tbch(0), op=ALU.max)     # ix1
                nc.vector.tensor_tensor(GB[:], pbch(2), tbch(2), op=ALU.min)     # ix2
                nc.vector.tensor_tensor(GC[:], GB[:], GA[:], op=ALU.subtract)    # wx
                gcf = GC[:].rearrange("p r j s -> p (r j s)")
                nc.vector.tensor_scalar(gcf, gcf, 0.0, None, op0=ALU.max)        # relu
                nc.vector.tensor_tensor(GD[:], pbch(1), tbch(1), op=ALU.max)     # iy1
                nc.vector.tensor_tensor(GA[:], pbch(3), tbch(3), op=ALU.min)     # iy2
                nc.vector.tensor_tensor(GB[:], GA[:], GD[:], op=ALU.subtract)    # wy
                gbf = GB[:].rearrange("p r j s -> p (r j s)")
                nc.vector.tensor_scalar(gbf, gbf, 0.0, None, op0=ALU.max)        # relu
                nc.vector.tensor_tensor(GD[:], GC[:], GB[:], op=ALU.mult)        # inter
                A12 = pp.tile(GRIDH, bf16, tag="a12", bufs=2)
                tabb = TAB[:, :, ssl].unsqueeze(1).broadcast_to(GRIDH)
                aeng.tensor_tensor(A12[:], pbch(4), tabb, op=ALU.add)            # pa + ta
                nc.vector.tensor_tensor(GC[:], A12[:], GD[:], op=ALU.subtract)   # den bf16

                # reciprocal quarters: Scalar upcast(+1e-6) -> DVE rcp -> Scalar downcast
                for hq in range(2):
                    qsl = slice(hq * SQ, (hq + 1) * SQ)
                    osl = slice(h * SH + hq * SQ, h * SH + (hq + 1) * SQ)
                    DENF = pp.tile([P, RCH, J, SQ], f32, tag="denf", bufs=2)
                    nc.scalar.activation(DENF[:], GC[:, :, :, qsl], AF.Copy, bias=1e-6)
                    RCPF = pp.tile([P, RCH, J, SQ], f32, tag="rcpf", bufs=2)
                    nc.vector.reciprocal_approx_fast(
                        RCPF[:].rearrange("p r j s -> p (r j s)"),
                        DENF[:].rearrange("p r j s -> p (r j s)"))
                    RCPB = pp.tile([P, RCH, J, SQ], bf16, tag="rcpb", bufs=2)
                    nc.scalar.copy(RCPB[:], RCPF[:])
                    nc.vector.tensor_tensor(IOUF[:, :, :, osl], GD[:, :, :, qsl], RCPB[:], op=ALU.mult)

                # BEST via tree-max over j for this half (fills rcp-wait gaps)
                cur = IOUF[:, :, :, ssl]
                width = J
                while width > 2:
                    half = width // 2
                    nt = pp.tile([P, RCH, half, SH], bf16, tag=f"bmh{half}")
                    nc.vector.tensor_tensor(nt[:], cur[:, :, 0:half], cur[:, :, half:width], op=ALU.max)
                    cur = nt[:]
                    width = half
                nc.vector.tensor_tensor(BEST[:, :, ssl], cur[:, :, 0], cur[:, :, 1], op=ALU.max)

        # conf softplus assembly
        MXN = sb.tile([P, ROWS], bf16, tag="mxn")   # max(-x,0) = |x| - max(x,0)
        nc.vector.tensor_tensor(MXN[:], AXB[:], MX0[:], op=ALU.subtract)
        nc.vector.tensor_tensor(SPP[:].rearrange("p s r -> p (s r)"), L1S[:], MX0[:], op=ALU.add)
        nc.vector.tensor_tensor(SPN[:].rearrange("p s r -> p (s r)"), L1S[:], MXN[:], op=ALU.add)

        nc.vector.tensor_scalar(MR[:].rearrange("p r s -> p (r s)"),
                                BEST[:].rearrange("p r s -> p (r s)"), 0.5, None, op0=ALU.is_gt)

        # ---------- fallback-mask chain (PE/Scalar parts overlap gather) ----------
        pst = ctx.enter_context(tc.tile_pool(name="pst", bufs=1, space="PSUM"))
        B4 = sb.tile([P, 4, S], bf16, tag="b4")
        nc.vector.tensor_tensor(B4[:], BEST[:, 0:4], BEST[:, 4:8], op=ALU.max)
        B2 = sb.tile([P, 2, S], bf16, tag="b2")
        nc.vector.tensor_tensor(B2[:], B4[:, 0:2], B4[:, 2:4], op=ALU.max)
        BESTS16 = sb.tile([P, S], bf16, tag="bests16")
        nc.vector.tensor_tensor(BESTS16[:], B2[:, 0], B2[:, 1], op=ALU.max)
        trb = pst.tile([S, P], bf16, tag="tp128")
        nc.tensor.transpose(trb[:], BESTS16[:], IDENTB[:])
        TB = sb.tile([S, P], f32, tag="tb")
        nc.scalar.copy(TB[:], trb[:])
        GMAX16 = sb.tile([S, 1], f32, tag="gmax16")
        nc.vector.tensor_reduce(GMAX16[:], TB[:], axis=AX.X, op=ALU.max)
        EQT = sb.tile([S, P], f32, tag="eqt")
        nc.vector.tensor_scalar(EQT[:], TB[:], GMAX16[:], None, op0=ALU.is_equal)
        NAFT = sb.tile([S, 1], f32, tag="naft")
        nc.vector.tensor_scalar(NAFT[:], GMAX16[:], 0.5, None, op0=ALU.is_le)
        NF128 = sb.tile([S, P], f32, tag="nf128")
        nc.vector.tensor_scalar(NF128[:], TB[:], 0.0, NAFT[:], op0=ALU.mult, op1=ALU.add)
        teqc = pst.tile([P, S], f32, tag="tpb")
        nc.tensor.transpose(teqc[:], EQT[:], IDENT[:S, :S])
        EQC = sb.tile([P, S], bf16, tag="eqc")
        nc.scalar.copy(EQC[:], teqc[:])
        tnaf = pst.tile([P, S], f32, tag="tpc")
        nc.tensor.transpose(tnaf[:], NF128[:], IDENT[:S, :S])
        NAFC = sb.tile([P, S], bf16, tag="nafc")
        nc.scalar.copy(NAFC[:], tnaf[:])
        ECN = sb.tile([P, S], bf16, tag="ecn")
        nc.vector.tensor_tensor(ECN[:], EQC[:], NAFC[:], op=ALU.mult)

        # ---------- eq + gather (cls first so CE label prep starts early) ----------
        GRID = [P, RCH, J, S]

        def tbc(q):
            return BT5T[:, q].unsqueeze(1).broadcast_to(GRID)

        with tc.tile_pool(name="tg", bufs=1) as tg:
            EQ = tg.tile(GRID, bf16, tag="eq")
            best_b = BEST[:].unsqueeze(2).broadcast_to(GRID)
            nc.vector.tensor_tensor(EQ[:], IOUF[:], best_b, op=ALU.is_equal)

            GT8 = tg.tile([P, 5, RCH, 8, S], bf16, tag="gt8")
            # Pool gathers q=3 end-to-end
            if POOL_GATHER:
                gpp = tg.tile(GRID, bf16, tag="gpp")
                nc.gpsimd.tensor_tensor(gpp[:], EQ[:], tbc(3), op=ALU.mult)
                g32p = tg.tile([P, RCH, 32, S], bf16, tag="g32p")
                nc.gpsimd.tensor_tensor(g32p[:], gpp[:, :, 0:32], gpp[:, :, 32:64], op=ALU.add)
                g16p = tg.tile([P, RCH, 16, S], bf16, tag="g16p")
                nc.gpsimd.tensor_tensor(g16p[:], g32p[:, :, 0:16], g32p[:, :, 16:32], op=ALU.add)
                nc.gpsimd.tensor_tensor(GT8[:, 3], g16p[:, :, 0:8], g16p[:, :, 8:16], op=ALU.add)

            qlist = [4, 0, 1] + ([] if POOL_GATHER else [3]) + [2]
            for q in qlist:
                gp = tg.tile(GRID, bf16, tag="gp", bufs=1)
                nc.vector.tensor_tensor(gp[:], EQ[:], tbc(q), op=ALU.mult)
                g32 = tg.tile([P, RCH, 32, S], bf16, tag="g32")
                nc.vector.tensor_tensor(g32[:], gp[:, :, 0:32], gp[:, :, 32:64], op=ALU.add)
                g16 = tg.tile([P, RCH, 16, S], bf16, tag="g16")
                nc.vector.tensor_tensor(g16[:], g32[:, :, 0:16], g32[:, :, 16:32], op=ALU.add)
                nc.vector.tensor_tensor(GT8[:, q], g16[:, :, 0:8], g16[:, :, 8:16], op=ALU.add)
                if q == 4:
                    # finish cls tail immediately: 8 -> 1, then label prep
                    c4 = tg.tile([P, RCH, 4, S], bf16, tag="c4")
                    nc.vector.tensor_tensor(c4[:], GT8[:, 4, :, 0:4], GT8[:, 4, :, 4:8], op=ALU.add)
                    c2 = tg.tile([P, RCH, 2, S], bf16, tag="c2")
                    nc.vector.tensor_tensor(c2[:], c4[:, :, 0:2], c4[:, :, 2:4], op=ALU.add)
                    nc.vector.tensor_tensor(MTC[:], c2[:, :, 0], c2[:, :, 1], op=ALU.add)
                    LBLC = tg.tile([P, RCH, S], bf16, tag="lblc")
                    nc.vector.tensor_scalar(LBLC[:].rearrange("p r s -> p (r s)"),
                                            MTC[:].rearrange("p r s -> p (r s)"),
                                            0.0, float(NCLS - 1), op0=ALU.max, op1=ALU.min)
                    nc.scalar.copy(LBLT[:], LBLC[:].rearrange("p r s -> p s r"))

            # shared box tail: 8 -> 4 -> 2 -> 1 over the 4 coord fields
            bt4 = GT8[:, 0:4].rearrange("p q r j s -> p (q r) j s")
            t84 = tg.tile([P, 4 * RCH, 4, S], bf16, tag="t84")
            nc.vector.tensor_tensor(t84[:], bt4[:, :, 0:4], bt4[:, :, 4:8], op=ALU.add)
            t42 = tg.tile([P, 4 * RCH, 2, S], bf16, tag="t42")
            nc.vector.tensor_tensor(t42[:], t84[:, :, 0:2], t84[:, :, 2:4], op=ALU.add)
            nc.vector.tensor_tensor(MT4[:].rearrange("p q r s -> p (q r) s"),
                                    t42[:, :, 0], t42[:, :, 1], op=ALU.add)

            # ---------- CE (4 chunks, Scalar prefetch overlaps gather DVE work) ----------
            HR = ROWS // 4
            SP2 = sb.tile([P, 2, ROWS], f32, tag="sp2")   # [:,0]=sumexp, [:,1]=pick
            with tc.tile_pool(name="cp", bufs=1) as cp:
                for ch in range(4):
                    rs = slice(ch * (S // 4), (ch + 1) * (S // 4))
                    fs = slice(ch * HR, (ch + 1) * HR)
                    LBLR = cp.tile([P, HR, NC80], bf16, tag="lblr", bufs=2)
                    nc.scalar.copy(LBLR[:], LBLT[:].rearrange("p s r -> p (s r)")[:, fs]
                                   .unsqueeze(2).broadcast_to([P, HR, NC80]))
                    EP = cp.tile([P, 2, HR, NC80], bf16, tag="ep", bufs=2)  # [e2 | pick]
                    nc.vector.memset(EP[:, 0, :, NCLS:NC80], 0.0)
                    logits = PRED[:, rs, :, 6:].rearrange("p s r c -> p (s r) c")
                    nc.scalar.activation(EP[:, 0, :, 0:NCLS], logits, AF.Exp)
                    ohc = cp.tile([P, HR, NC80], bf16, tag="ohc", bufs=2)
                    iot_b = IOTA80[:].unsqueeze(1).broadcast_to([P, HR, NC80])
                    nc.vector.tensor_tensor(ohc[:], iot_b, LBLR[:], op=ALU.is_equal)
                    nc.vector.tensor_tensor(EP[:, 1], ohc[:], EP[:, 0], op=ALU.mult)
                    # joint tree: 80 -> 40 -> 20 -> 10 -> 5 -> reduce
                    cur = EP[:]
                    width = NC80
                    while width > 5:
                        half = width // 2
                        nt = cp.tile([P, 2, HR, half], bf16, tag=f"se{half}", name="nt")
                        nc.vector.tensor_tensor(nt[:], cur[:, :, :, 0:half], cur[:, :, :, half:width], op=ALU.add)
                        cur = nt[:]
                        width = half
                    nc.vector.tensor_reduce(SP2[:, :, fs], cur, axis=AX.X, op=ALU.add)


        # ---------- smooth L1 ((r, s) layout; x2 folded into transpose scale) ----------
        SL1T = sb.tile([P, S, RCH], bf16, tag="sl1t")
        with tc.tile_pool(name="sp", bufs=1) as sp:
            d = sp.tile([P, 4, RCH, S], bf16, tag="d")
            nc.vector.tensor_tensor(d[:], PB[:, 0:4], MT4[:], op=ALU.subtract)
            ad = sp.tile([P, 4, RCH, S], bf16, tag="ad")
            nc.scalar.activation(ad[:], d[:], AF.Abs)                        # |d|
            tmh = sp.tile([P, 4, RCH, S], bf16, tag="tmh")
            nc.vector.tensor_scalar(tmh[:].rearrange("p q r s -> p (q r s)"),
                                    ad[:].rearrange("p q r s -> p (q r s)"),
                                    1.0, 0.5, op0=ALU.min, op1=ALU.mult)     # min(|d|,1)/2
            uu = sp.tile([P, 4, RCH, S], bf16, tag="uu")
            nc.vector.tensor_tensor(uu[:], ad[:], tmh[:], op=ALU.subtract)   # |d| - tm/2
            sl1h = sp.tile([P, 4, RCH, S], bf16, tag="sl1h")
            nc.vector.tensor_tensor(sl1h[:], tmh[:], uu[:], op=ALU.mult)     # sl1/2
            q2 = sp.tile([P, 2, RCH, S], bf16, tag="q2")
            nc.vector.tensor_tensor(q2[:], sl1h[:, 0:2], sl1h[:, 2:4], op=ALU.add)
            shs = sp.tile([P, RCH, S], bf16, tag="shs")
            nc.vector.tensor_tensor(shs[:], q2[:, 0], q2[:, 1], op=ALU.add)
            nc.scalar.activation(SL1T[:], shs[:].rearrange("p r s -> p s r"),
                                 AF.Copy, scale=2.0)                          # x2 fold

        # ---------- final match mask + weighted sums ----------
        EQB = sb.tile([P, RCH, S], bf16, tag="eqb")
        nc.vector.tensor_tensor(EQB[:], BEST[:],
                                BESTS16[:].unsqueeze(1).broadcast_to([P, RCH, S]), op=ALU.is_equal)
        M2 = sb.tile([P, RCH, S], bf16, tag="m2")
        nc.vector.tensor_tensor(M2[:], EQB[:],
                                ECN[:].unsqueeze(1).broadcast_to([P, RCH, S]), op=ALU.mult)
        MM = sb.tile([P, RCH, S], bf16, tag="mm")
        nc.vector.tensor_tensor(MM[:], MR[:], M2[:], op=ALU.add)

        FQ = sb.tile([P, 6, S, RCH], bf16, tag="fq")
        nc.scalar.copy(FQ[:, 0], MM[:].rearrange("p r s -> p s r"))
        nc.vector.tensor_tensor(FQ[:, 1], FQ[:, 0], SL1T[:], op=ALU.mult)
        nc.vector.tensor_tensor(FQ[:, 3], FQ[:, 0], SPN[:], op=ALU.mult)
        nc.vector.tensor_tensor(FQ[:, 4], FQ[:, 0], SPP[:], op=ALU.mult)
        nc.vector.tensor_copy(FQ[:, 5], SPP[:])

        # ---------- partition sums via ones-matmul (q=2 deferred until CE) ----------
        R768 = sb.tile([1, 6, S, RCH], f32, tag="r768")
        fqf = FQ[:].rearrange("p q s r -> p (q s r)")
        psr = ctx.enter_context(tc.tile_pool(name="psr", bufs=1, space="PSUM"))
        for lo, hi in ((0, 256), (384, 768)):
            rq_ps = psr.tile([1, 384], f32, tag="rq_ps", bufs=3, name=f"rq{lo}")
            nc.tensor.matmul(rq_ps[:, 0:hi - lo], ONESB[:], fqf[:, lo:hi], start=True, stop=True)
            nc.vector.tensor_copy(R768[:].rearrange("o q s r -> o (q s r)")[:, lo:hi], rq_ps[:, 0:hi - lo])

        # per-sample scalars that don't depend on CE: compute before CE finishes
        RQ = sb.tile([1, 6, S], f32, tag="rq")
        for q in (0, 1, 3, 4, 5):
            nc.vector.tensor_reduce(RQ[:, q], R768[:, q], axis=AX.X, op=ALU.add)
        mcnt = RQ[:, 0]; bbox_n = RQ[:, 1]
        spn_n = RQ[:, 3]; spp_m = RQ[:, 4]; spp_all = RQ[:, 5]

        def t16(tag):
            return sb.tile([1, S], f32, tag=tag, name=tag)

        d4 = t16("d4"); nc.vector.tensor_scalar(d4[:], mcnt, 4.0, 1.0, op0=ALU.mult, op1=ALU.max)
        r4 = t16("r4"); nc.vector.reciprocal(r4[:], d4[:])
        bbox = t16("bbox"); nc.vector.tensor_tensor(bbox[:], bbox_n, r4[:], op=ALU.mult)
        d1 = t16("d1"); nc.vector.tensor_scalar(d1[:], mcnt, 1.0, None, op0=ALU.max)
        r1 = t16("r1"); nc.vector.reciprocal(r1[:], d1[:])
        confm = t16("confm"); nc.vector.tensor_tensor(confm[:], spn_n, r1[:], op=ALU.mult)
        ucnt = t16("ucnt"); nc.vector.tensor_scalar(ucnt[:], mcnt, -1.0, float(N), op0=ALU.mult, op1=ALU.add)
        du = t16("du"); nc.vector.tensor_scalar(du[:], ucnt[:], 1.0, None, op0=ALU.max)
        ru = t16("ru"); nc.vector.reciprocal(ru[:], du[:])
        cun = t16("cun"); nc.vector.tensor_tensor(cun[:], spp_all, spp_m, op=ALU.subtract)
        confu = t16("confu"); nc.vector.tensor_tensor(confu[:], cun[:], ru[:], op=ALU.mult)
        csum = t16("csum"); nc.vector.tensor_tensor(csum[:], confm[:], confu[:], op=ALU.add)
        chalf = t16("chalf"); nc.vector.tensor_scalar(chalf[:], csum[:], 0.5, None, op0=ALU.mult)
        ug = t16("ug"); nc.vector.tensor_scalar(ug[:], ucnt[:], 0.0, None, op0=ALU.is_gt)
        ugn = t16("ugn"); nc.vector.tensor_scalar(ugn[:], ucnt[:], 0.0, None, op0=ALU.is_le)
        c1 = t16("c1"); nc.vector.tensor_tensor(c1[:], chalf[:], ug[:], op=ALU.mult)
        c2 = t16("c2"); nc.vector.tensor_tensor(c2[:], confm[:], ugn[:], op=ALU.mult)
        confL = t16("confL"); nc.vector.tensor_tensor(confL[:], c1[:], c2[:], op=ALU.add)
        lnv = t16("lnv"); nc.vector.tensor_scalar(lnv[:], spp_all, 1.0 / float(N), None, op0=ALU.mult)
        kvg = t16("kvg"); nc.vector.tensor_scalar(kvg[:], KV16[:], 0.0, None, op0=ALU.is_gt)
        kvn = t16("kvn"); nc.vector.tensor_scalar(kvn[:], KV16[:], 0.0, None, op0=ALU.is_le)
        bc = t16("bc"); nc.vector.tensor_tensor(bc[:], bbox[:], confL[:], op=ALU.add)

        LL2 = sb.tile([P, 2, ROWS], f32, tag="ll2")
        nc.scalar.activation(LL2[:], SP2[:], AF.Ln)
        CET = sb.tile([P, S, RCH], bf16, tag="cet")
        nc.vector.tensor_tensor(CET[:], LL2[:, 0].rearrange("p (s r) -> p s r", s=S),
                                LL2[:, 1].rearrange("p (s r) -> p s r", s=S), op=ALU.subtract)

        nc.vector.tensor_tensor(FQ[:, 2], FQ[:, 0], CET[:], op=ALU.mult)
        rq_ps2 = psr.tile([1, 384], f32, tag="rq_ps", bufs=3, name="rq2")
        nc.tensor.matmul(rq_ps2[:, 0:128], ONESB[:], fqf[:, 256:384], start=True, stop=True)
        nc.vector.tensor_copy(R768[:].rearrange("o q s r -> o (q s r)")[:, 256:384], rq_ps2[:, 0:128])

        # ---------- final: CE-dependent tail ----------
        nc.vector.tensor_reduce(RQ[:, 2], R768[:, 2], axis=AX.X, op=ALU.add)
        cls_n = RQ[:, 2]
        clsl = t16("clsl"); nc.vector.tensor_tensor(clsl[:], cls_n, r1[:], op=ALU.mult)
        lv = t16("lv"); nc.vector.tensor_tensor(lv[:], bc[:], clsl[:], op=ALU.add)
        lA = t16("lA"); nc.vector.tensor_tensor(lA[:], lv[:], kvg[:], op=ALU.mult)
        lB = t16("lB"); nc.vector.tensor_tensor(lB[:], lnv[:], kvn[:], op=ALU.mult)
        LROW = t16("lrow"); nc.vector.tensor_tensor(LROW[:], lA[:], lB[:], op=ALU.add)
        nc.sync.dma_start(loss_d[:], LROW[:])

    lp.__exit__(None, None, None)
    return preds_d, tgts_d, loss_d


_NC_CACHE = {}


def get_nc():
    if "nc" not in _NC_CACHE:
        nc = bacc.Bacc("TRN2", target_bir_lowering=False, debug=False)
        build_kernel(nc)
        nc.compile()
        _NC_CACHE["nc"] = nc
    return _NC_CACHE["nc"]


def kernel(preds: np.ndarray, targets: np.ndarray) -> np.ndarray:
    from concourse.bass_utils import run_bass_kernel_spmd

    nc = get_nc()
    in_maps = []
    for c in range(NCORES):
        in_maps.append({
            "preds": np.ascontiguousarray(preds[c * S:(c + 1) * S], dtype=np.float32),
            "tgts": np.ascontiguousarray(targets[c * S:(c + 1) * S], dtype=np.float32),
        })
    res = run_bass_kernel_spmd(nc, in_maps, core_ids=list(range(NCORES)))
    per_sample = np.concatenate([res.results[c]["loss"].reshape(-1) for c in range(NCORES)])
    return np.float32(per_sample.sum() / B)


# revision 24
# speedup vs baseline: 1.2069x; 1.1897x over previous
"""Trainium2 Bass kernel for nn_DetectionLoss (B=128, N=1024, MAX_T=64, 80 classes).

Contract: kernel(**inputs) takes FULL inputs {preds: (128,1024,85) f32,
targets: (128,64,5) f32} and returns the FULL scalar output (f32 (),
mean of per-sample losses), computed data-parallel on 8 NeuronCores
(16 samples per core).

v4 design (vs v2 baseline at ~207us, v3 at ~211us):
- Grid layout [P, r, j, s] with s innermost: broadcasts are stride-0 MIDDLE
  dims, so every DVE tensor_tensor runs in 2x mode with zero replicate copies.
- Targets broadcast via 10 ones-matmuls into batched PSUM, evacuated with
  TWO large Scalar copies (v3 did 10 small ones, latency-serialized).
- Invalid targets: x1=x2=0 via vb-multiply on Pool (inter==0, never wins).
- Pair phase in 2 sample-halves; Pool computes a12 = pa+ta concurrently.
- Division: den bf16 (2x) -> Scalar upcast +1e-6 bias -> DVE
  reciprocal_approx_fast (only 1x op) -> Scalar downcast, 4-quarter pipeline.
- Gather: eq is_eq at 2x; field q=3 gathered fully on Pool, rest on DVE.
- CE: Scalar exp/label-broadcast lead DVE trees; pick multiply on Pool.
- Fallback-mask chain placed right after BEST so its PE transposes +
  Scalar copies overlap gather.
"""
import numpy as np

import concourse.bass as bass
import concourse.bacc as bacc
import concourse.mybir as mybir
import concourse.tile as tile
from contextlib import ExitStack

f32 = mybir.dt.float32
bf16 = mybir.dt.bfloat16
i32 = mybir.dt.int32
AF = mybir.ActivationFunctionType
ALU = mybir.AluOpType
AX = mybir.AxisListType

# problem constants (hardcoded per spec)
B, N, MAX_T, PD = 128, 1024, 64, 85
NCLS = 79              # logits are pred[:, 6:85]
NC80 = 80              # padded class width
NCORES = 8
S = B // NCORES        # 16 samples per core
P = 128                # partitions
RCH = N // P           # 8 preds per partition per sample
J = MAX_T

ROWS = S * RCH         # 128 pred rows per partition, (s, r) order
SH = S // 2            # pair-phase half
GRIDH = [P, RCH, J, SH]
POOL_MASK = False       # gpsimd masks invalid-target x planes
POOL_A12 = False        # gpsimd computes a12 = pa + ta
POOL_GATHER = False     # gpsimd gathers field q=3 end-to-end
POOL_PM = False         # gpsimd computes CE pick multiply


def build_kernel(nc):
    preds_d = nc.dram_tensor("preds", [S, N, PD], f32, kind="ExternalInput")
    tgts_d = nc.dram_tensor("tgts", [S, MAX_T, 5], f32, kind="ExternalInput")
    loss_d = nc.dram_tensor("loss", [1, S], f32, kind="ExternalOutput")

    lp = nc.allow_low_precision("bf16 pipeline validated numerically vs reference (sim rel ~1.5e-3)")
    lp.__enter__()

    with tile.TileContext(nc) as tc, ExitStack() as ctx:
        sb = ctx.enter_context(tc.tile_pool(name="sb", bufs=1))

        # ---------- constants ----------
        ones_col = sb.tile([1, P], f32, tag="ones_col")       # lhsT (K=1, M=128)
        nc.vector.memset(ones_col[:], 1.0)
        ones_colb = sb.tile([1, P], bf16, tag="ones_colb")    # bf16 lhsT
        nc.vector.memset(ones_colb[:], 1.0)
        ONESB = sb.tile([P, 1], bf16, tag="onesb")            # lhsT for column sums
        nc.vector.memset(ONESB[:], 1.0)
        iot80_i = sb.tile([P, NC80], i32, tag="iot80_i")
        nc.gpsimd.iota(iot80_i[:], pattern=[[1, NC80]], base=0, channel_multiplier=0)
        IOTA80 = sb.tile([P, NC80], bf16, tag="iota80")
        nc.vector.tensor_copy(IOTA80[:], iot80_i[:])
        idn_i = sb.tile([P, P], i32, tag="idn_i")
        nc.gpsimd.iota(idn_i[:], pattern=[[1, P]], base=0, channel_multiplier=-1)
        IDENT = sb.tile([P, P], f32, tag="ident")
        nc.vector.tensor_scalar(IDENT[:], idn_i[:], 0, None, op0=ALU.is_equal)
        IDENTB = sb.tile([P, P], bf16, tag="identb")
        nc.vector.tensor_copy(IDENTB[:], IDENT[:])

        # ---------- persistent tiles ----------
        PRED = sb.tile([P, S, RCH, PD], f32, tag="pred")      # 43.5 KB
        BT5T = sb.tile([P, 5, J, S], bf16, tag="bt5t")        # targets, (j, s) layout
        PB = sb.tile([P, 5, RCH, S], bf16, tag="pb")          # pred x1,y1,x2,y2,area (r, s)
        IOUF = sb.tile([P, RCH, J, S], bf16, tag="iouf")      # 16 KB
        KV16 = sb.tile([1, S], f32, tag="kv16")               # valid-target count / sample
        BEST = sb.tile([P, RCH, S], bf16, tag="best")
        MR = sb.tile([P, RCH, S], bf16, tag="mr")
        MTC = sb.tile([P, RCH, S], bf16, tag="mtc")           # gathered cls
        MT4 = sb.tile([P, 4, RCH, S], bf16, tag="mt4")        # gathered box
        LBLT = sb.tile([P, S, RCH], bf16, tag="lblt")         # clipped label, (s, r)
        TAB = sb.tile([P, J, S], bf16, tag="tab")             # target areas
        L1S = sb.tile([P, ROWS], bf16, tag="l1s")             # ln(1+e^-|x|)
        MX0 = sb.tile([P, ROWS], bf16, tag="mx0")             # max(x,0)
        AXB = sb.tile([P, ROWS], bf16, tag="axbp")            # |x|
        SPP = sb.tile([P, S, RCH], bf16, tag="spp")
        SPN = sb.tile([P, S, RCH], bf16, tag="spn")

        # ---------- loads + target prep + pred extraction ----------
        with tc.tile_pool(name="ldp", bufs=1) as ldp:
            TROW = ldp.tile([1, S, MAX_T, 5], f32, tag="trow")
            nc.sync.dma_start(TROW[:], tgts_d[:].rearrange("s t c -> (s t c)").unsqueeze(0))
            for s8 in range(8):
                sl8 = slice(s8 * 2, (s8 + 1) * 2)
                src = preds_d[sl8].rearrange("s (p r) q -> p s r q", p=P)
                nc.sync.dma_start(PRED[:, sl8], src)

            # transpose target planes to (q, j, s) bf16: Scalar does 4,1,3 while
            # DVE does 0,2 (+ masking), minimizing the critical path to the matmuls
            TROWT = ldp.tile([1, 6, J, S], bf16, tag="trowt")
            nc.scalar.copy(TROWT[0:1, 4], TROW[0:1, :, :, 4].rearrange("o s j -> o j s"))
            nc.scalar.copy(TROWT[0:1, 1], TROW[0:1, :, :, 1].rearrange("o s j -> o j s"))
            nc.scalar.copy(TROWT[0:1, 3], TROW[0:1, :, :, 3].rearrange("o s j -> o j s"))
            VB0T = ldp.tile([1, J, S], bf16, tag="vb0t")
            nc.vector.tensor_scalar(VB0T[:].rearrange("o j s -> o (j s)"),
                                    TROWT[0:1, 4].rearrange("o j s -> o (j s)"),
                                    0.0, None, op0=ALU.is_ge)
            nc.vector.tensor_tensor(TROWT[0:1, 0], TROW[0:1, :, :, 0].rearrange("o s j -> o j s"),
                                    VB0T[:], op=ALU.mult)
            nc.vector.tensor_tensor(TROWT[0:1, 2], TROW[0:1, :, :, 2].rearrange("o s j -> o j s"),
                                    VB0T[:], op=ALU.mult)
            TWH = ldp.tile([1, 2, J, S], bf16, tag="twh")
            nc.vector.tensor_tensor(TWH[:], TROWT[0:1, 2:4], TROWT[0:1, 0:2], op=ALU.subtract)
            nc.vector.tensor_tensor(TROWT[0:1, 5], TWH[0:1, 0], TWH[0:1, 1], op=ALU.mult)
            # kv count per sample: strided reduce over j
            nc.vector.tensor_reduce(KV16[:], VB0T[:].rearrange("o j s -> o s j"), axis=AX.X, op=ALU.add)

            # pred fields transposed to (r, s), per DMA chunk
            for s8 in range(4):
                sl8 = slice(s8 * 4, (s8 + 1) * 4)
                for q in range(4):
                    nc.scalar.copy(PB[:, q, :, sl8], PRED[:, sl8, :, q].rearrange("p s r -> p r s"))

            # broadcast to all partitions via ones-matmuls, batched PSUM
            with tc.tile_pool(name="psa", bufs=1, space="PSUM") as psa:
                btA = psa.tile([P, 4 * J * S], f32, tag="bta")   # 16 KB: planes 0-3
                trf = TROWT[:].rearrange("o q j s -> o (q j s)")
                for h in range(8):
                    nc.tensor.matmul(btA[:, h * 512:(h + 1) * 512],
                                     ones_colb[:], trf[:, h * 512:(h + 1) * 512],
                                     start=True, stop=True)
                btv = btA[:].rearrange("p (q j s) -> p q j s", q=4, j=J)
                nc.scalar.copy(BT5T[:, 0:2], btv[:, 0:2])
                nc.scalar.copy(BT5T[:, 2:4], btv[:, 2:4])

            with tc.tile_pool(name="psb", bufs=1, space="PSUM") as psb:
                btB = psb.tile([P, 2 * J * S], f32, tag="btb")   # planes 4 (cls), 5 (area)
                for h in range(4):
                    nc.tensor.matmul(btB[:, h * 512:(h + 1) * 512],
                                     ones_colb[:], trf[:, 4096 + h * 512:4096 + (h + 1) * 512],
                                     start=True, stop=True)
                nc.scalar.copy(BT5T[:, 4], btB[:, 0:J * S].rearrange("p (j s) -> p j s", j=J))
                nc.scalar.copy(TAB[:], btB[:, J * S:2 * J * S].rearrange("p (j s) -> p j s", j=J))

            # conf softplus pieces on Scalar: sp(x) = ln(1+e^-|x|) + max(x,0)
            CFV = PRED[:, :, :, 4].rearrange("p s r -> p (s r)")
            AXC = ldp.tile([P, ROWS], f32, tag="axc")
            nc.scalar.activation(AXC[:], CFV, AF.Abs)
            EN = ldp.tile([P, ROWS], f32, tag="en")
            nc.scalar.activation(EN[:], AXC[:], AF.Exp, scale=-1.0)
            nc.scalar.activation(L1S[:], EN[:], AF.Ln, bias=1.0)
            nc.scalar.activation(MX0[:], CFV, AF.Relu)
            nc.scalar.copy(AXB[:], AXC[:])

        # ---------- pair phase in s-halves ----------
        with tc.tile_pool(name="pp", bufs=1) as pp:
            aeng = nc.gpsimd if POOL_A12 else nc.vector
            SQ = S // 4
            for h in range(2):
                ssl = slice(h * SH, (h + 1) * SH)

                def pbch(q):
                    return PB[:, q, :, ssl].unsqueeze(2).broadcast_to(GRIDH)

                def tbch(q):
                    return BT5T[:, q, :, ssl].unsqueeze(1).broadcast_to(GRIDH)

                GA = pp.tile(GRIDH, bf16, tag="ga")
                GB = pp.tile(GRIDH, bf16, tag="gb")
                GC = pp.tile(GRIDH, bf16, tag="gc")
                GD = pp.tile(GRIDH, bf16, tag="gd")
                # pred areas for this half
                PWH = pp.tile([P, RCH, SH], bf16, tag="pwh")
                nc.vector.tensor_tensor(PWH[:], PB[:, 2, :, ssl], PB[:, 0, :, ssl], op=ALU.subtract)
                PHH = pp.tile([P, RCH, SH], bf16, tag="phh")
                nc.vector.tensor_tensor(PHH[:], PB[:, 3, :, ssl], PB[:, 1, :, ssl], op=ALU.subtract)
                nc.vector.tensor_tensor(PB[:, 4, :, ssl], PWH[:], PHH[:], op=ALU.mult)
                nc.vector.tensor_tensor(GA[:], pbch(0), tbch(0), op=ALU.max)     # ix1
                nc.vector.tensor_tensor(GB[:], pbch(2), tbch(2), op=ALU.min)     # ix2
                nc.vector.tensor_tensor(GC[:], GB[:], GA[:], op=ALU.subtract)    # wx
                gcf = GC[:].rearrange("p r j s -> p (r j s)")
                nc.vector.tensor_scalar(gcf, gcf, 0.0, None, op0=ALU.max)        # relu
                nc.vector.tensor_tensor(GD[:], pbch(1), tbch(1), op=ALU.max)     # iy1
                nc.vector.tensor_tensor(GA[:], pbch(3), tbch(3), op=ALU.min)     # iy2
                nc.vector.tensor_tensor(GB[:], GA[:], GD[:], op=ALU.subtract)    # wy
                gbf = GB[:].rearrange("p r j s -> p (r j s)")
                nc.vector.tensor_scalar(gbf, gbf, 0.0, None, op0=ALU.max)        # relu
                nc.vector.tensor_tensor(GD[:], GC[:], GB[:], op=ALU.mult)        # inter
                A12 = pp.tile(GRIDH, bf16, tag="a12", bufs=2)
                tabb = TAB[:, :, ssl].unsqueeze(1).broadcast_to(GRIDH)
                aeng.tensor_tensor(A12[:], pbch(4), tabb, op=ALU.add)            # pa + ta
                nc.vector.tensor_tensor(GC[:], A12[:], GD[:], op=ALU.subtract)   # den bf16

                # reciprocal quarters: Scalar upcast(+1e-6) -> DVE rcp -> Scalar downcast
                for hq in range(2):
                    qsl = slice(hq * SQ, (hq + 1) * SQ)
                    osl = slice(h * SH + hq * SQ, h * SH + (hq + 1) * SQ)
                    DENF = pp.tile([P, RCH, J, SQ], f32, tag="denf", bufs=2)
                    nc.scalar.activation(DENF[:], GC[:, :, :, qsl], AF.Copy, bias=1e-6)
                    RCPF = pp.tile([P, RCH, J, SQ], f32, tag="rcpf", bufs=2)
                    nc.vector.reciprocal_approx_fast(
                        RCPF[:].rearrange("p r j s -> p (r j s)"),
                        DENF[:].rearrange("p r j s -> p (r j s)"))
                    RCPB = pp.tile([P, RCH, J, SQ], bf16, tag="rcpb", bufs=2)
                    nc.scalar.copy(RCPB[:], RCPF[:])
                    nc.vector.tensor_tensor(IOUF[:, :, :, osl], GD[:, :, :, qsl], RCPB[:], op=ALU.mult)

                # BEST via tree-max over j for this half (fills rcp-wait gaps)
                cur = IOUF[:, :, :, ssl]
                width = J
                while width > 2:
                    half = width // 2
                    nt = pp.tile([P, RCH, half, SH], bf16, tag=f"bmh{half}")
                    nc.vector.tensor_tensor(nt[:], cur[:, :, 0:half], cur[:, :, half:width], op=ALU.max)
                    cur = nt[:]
                    width = half
                nc.vector.tensor_tensor(BEST[:, :, ssl], cur[:, :, 0], cur[:, :, 1], op=ALU.max)

        # conf softplus assembly
        MXN = sb.tile([P, ROWS], bf16, tag="mxn")   # max(-x,0) = |x| - max(x,0)
        nc.vector.tensor_tensor(MXN[:], AXB[:], MX0[:], op=ALU.subtract)
        nc.vector.tensor_tensor(SPP[:].rearrange("p s r -> p (s r)"), L1S[:], MX0[:], op=ALU.add)
        nc.vector.tensor_tensor(SPN[:].rearrange("p s r -> p (s r)"), L1S[:], MXN[:], op=ALU.add)

        nc.vector.tensor_scalar(MR[:].rearrange("p r s -> p (r s)"),
                                BEST[:].rearrange("p r s -> p (r s)"), 0.5, None, op0=ALU.is_gt)

        # ---------- fallback-mask chain (PE/Scalar parts overlap gather) ----------
        pst = ctx.enter_context(tc.tile_pool(name="pst", bufs=1, space="PSUM"))
        B4 = sb.tile([P, 4, S], bf16, tag="b4")
        nc.vector.tensor_tensor(B4[:], BEST[:, 0:4], BEST[:, 4:8], op=ALU.max)
        B2 = sb.tile([P, 2, S], bf16, tag="b2")
        nc.vector.tensor_tensor(B2[:], B4[:, 0:2], B4[:, 2:4], op=ALU.max)
        BESTS16 = sb.tile([P, S], bf16, tag="bests16")
        nc.vector.tensor_tensor(BESTS16[:], B2[:, 0], B2[:, 1], op=ALU.max)
        trb = pst.tile([S, P], bf16, tag="tp128")
        nc.tensor.transpose(trb[:], BESTS16[:], IDENTB[:])
        TB = sb.tile([S, P], f32, tag="tb")
        nc.scalar.copy(TB[:], trb[:])
        GMAX16 = sb.tile([S, 1], f32, tag="gmax16")
        nc.vector.tensor_reduce(GMAX16[:], TB[:], axis=AX.X, op=ALU.max)
        EQT = sb.tile([S, P], f32, tag="eqt")
        nc.vector.tensor_scalar(EQT[:], TB[:], GMAX16[:], None, op0=ALU.is_equal)
        NAFT = sb.tile([S, 1], f32, tag="naft")
        nc.vector.tensor_scalar(NAFT[:], GMAX16[:], 0.5, None, op0=ALU.is_le)
        NF128 = sb.tile([S, P], f32, tag="nf128")
        nc.vector.tensor_scalar(NF128[:], TB[:], 0.0, NAFT[:], op0=ALU.mult, op1=ALU.add)
        teqc = pst.tile([P, S], f32, tag="tpb")
        nc.tensor.transpose(teqc[:], EQT[:], IDENT[:S, :S])
        EQC = sb.tile([P, S], bf16, tag="eqc")
        nc.scalar.copy(EQC[:], teqc[:])
        tnaf = pst.tile([P, S], f32, tag="tpc")
        nc.tensor.transpose(tnaf[:], NF128[:], IDENT[:S, :S])
        NAFC = sb.tile([P, S], bf16, tag="nafc")
        nc.scalar.copy(NAFC[:], tnaf[:])
        ECN = sb.tile([P, S], bf16, tag="ecn")
        nc.vector.tensor_tensor(ECN[:], EQC[:], NAFC[:], op=ALU.mult)

        # ---------- eq + gather (cls first so CE label prep starts early) ----------
        GRID = [P, RCH, J, S]

        def tbc(q):
            return BT5T[:, q].unsqueeze(1).broadcast_to(GRID)

        with tc.tile_pool(name="tg", bufs=1) as tg:
            EQ = tg.tile(GRID, bf16, tag="eq")
            best_b = BEST[:].unsqueeze(2).broadcast_to(GRID)
            nc.vector.tensor_tensor(EQ[:], IOUF[:], best_b, op=ALU.is_equal)

            GT8 = tg.tile([P, 5, RCH, 8, S], bf16, tag="gt8")
            # Pool gathers q=3 end-to-end
            if POOL_GATHER:
                gpp = tg.tile(GRID, bf16, tag="gpp")
                nc.gpsimd.tensor_tensor(gpp[:], EQ[:], tbc(3), op=ALU.mult)
                g32p = tg.tile([P, RCH, 32, S], bf16, tag="g32p")
                nc.gpsimd.tensor_tensor(g32p[:], gpp[:, :, 0:32], gpp[:, :, 32:64], op=ALU.add)
                g16p = tg.tile([P, RCH, 16, S], bf16, tag="g16p")
                nc.gpsimd.tensor_tensor(g16p[:], g32p[:, :, 0:16], g32p[:, :, 16:32], op=ALU.add)
                nc.gpsimd.tensor_tensor(GT8[:, 3], g16p[:, :, 0:8], g16p[:, :, 8:16], op=ALU.add)

            qlist = [4, 0, 1] + ([] if POOL_GATHER else [3]) + [2]
            for q in qlist:
                gp = tg.tile(GRID, bf16, tag="gp", bufs=1)
                nc.vector.tensor_tensor(gp[:], EQ[:], tbc(q), op=ALU.mult)
                g32 = tg.tile([P, RCH, 32, S], bf16, tag="g32")
                nc.vector.tensor_tensor(g32[:], gp[:, :, 0:32], gp[:, :, 32:64], op=ALU.add)
                g16 = tg.tile([P, RCH, 16, S], bf16, tag="g16")
                nc.vector.tensor_tensor(g16[:], g32[:, :, 0:16], g32[:, :, 16:32], op=ALU.add)
                nc.vector.tensor_tensor(GT8[:, q], g16[:, :, 0:8], g16[:, :, 8:16], op=ALU.add)
                if q == 4:
                    # finish cls tail immediately: 8 -> 1, then label prep
                    c4 = tg.tile([P, RCH, 4, S], bf16, tag="c4")
                    nc.vector.tensor_tensor(c4[:], GT8[:, 4, :, 0:4], GT8[:, 4, :, 4:8], op=ALU.add)
                    c2 = tg.tile([P, RCH, 2, S], bf16, tag="c2")
                    nc.vector.tensor_tensor(c2[:], c4[:, :, 0:2], c4[:, :, 2:4], op=ALU.add)
                    nc.vector.tensor_tensor(MTC[:], c2[:, :, 0], c2[:, :, 1], op=ALU.add)
                    LBLC = tg.tile([P, RCH, S], bf16, tag="lblc")
                    nc.vector.tensor_scalar(LBLC[:].rearrange("p r s -> p (r s)"),
                                            MTC[:].rearrange("p r s -> p (r s)"),
                                            0.0, float(NCLS - 1), op0=ALU.max, op1=ALU.min)
                    nc.scalar.copy(LBLT[:], LBLC[:].rearrange("p r s -> p s r"))

            # shared box tail: 8 -> 4 -> 2 -> 1 over the 4 coord fields
            bt4 = GT8[:, 0:4].rearrange("p q r j s -> p (q r) j s")
            t84 = tg.tile([P, 4 * RCH, 4, S], bf16, tag="t84")
            nc.vector.tensor_tensor(t84[:], bt4[:, :, 0:4], bt4[:, :, 4:8], op=ALU.add)
            t42 = tg.tile([P, 4 * RCH, 2, S], bf16, tag="t42")
            nc.vector.tensor_tensor(t42[:], t84[:, :, 0:2], t84[:, :, 2:4], op=ALU.add)
            nc.vector.tensor_tensor(MT4[:].rearrange("p q r s -> p (q r) s"),
                                    t42[:, :, 0], t42[:, :, 1], op=ALU.add)

            # ---------- CE (4 chunks, Scalar prefetch overlaps gather DVE work) ----------
            HR = ROWS // 4
            SP2 = sb.tile([P, 2, ROWS], f32, tag="sp2")   # [:,0]=sumexp, [:,1]=pick
            with tc.tile_pool(name="cp", bufs=1) as cp:
                for ch in range(4):
                    rs = slice(ch * (S // 4), (ch + 1) * (S // 4))
                    fs = slice(ch * HR, (ch + 1) * HR)
                    LBLR = cp.tile([P, HR, NC80], bf16, tag="lblr", bufs=2)
                    nc.scalar.copy(LBLR[:], LBLT[:].rearrange("p s r -> p (s r)")[:, fs]
                                   .unsqueeze(2).broadcast_to([P, HR, NC80]))
                    EP = cp.tile([P, 2, HR, NC80], bf16, tag="ep", bufs=2)  # [e2 | pick]
                    nc.vector.memset(EP[:, 0, :, NCLS:NC80], 0.0)
                    logits = PRED[:, rs, :, 6:].rearrange("p s r c -> p (s r) c")
                    nc.scalar.activation(EP[:, 0, :, 0:NCLS], logits, AF.Exp)
                    ohc = cp.tile([P, HR, NC80], bf16, tag="ohc", bufs=2)
                    iot_b = IOTA80[:].unsqueeze(1).broadcast_to([P, HR, NC80])
                    nc.vector.tensor_tensor(ohc[:], iot_b, LBLR[:], op=ALU.is_equal)
                    nc.vector.tensor_tensor(EP[:, 1], ohc[:], EP[:, 0], op=ALU.mult)
                    # joint tree: 80 -> 40 -> 20 -> 10 -> 5 -> reduce
                    cur = EP[:]
                    width = NC80
                    while width > 5:
                        half = width // 2
                        nt = cp.tile([P, 2, HR, half], bf16, tag=f"se{half}", name="nt")
                        nc.vector.tensor_tensor(nt[:], cur[:, :, :, 0:half], cur[:, :, :, half:width], op=ALU.add)
                        cur = nt[:]
                        width = half
                    nc.vector.tensor_reduce(SP2[:, :, fs], cur, axis=AX.X, op=ALU.add)


        # ---------- smooth L1 ((r, s) layout; x2 folded into transpose scale) ----------
        SL1T = sb.tile([P, S, RCH], bf16, tag="sl1t")
        with tc.tile_pool(name="sp", bufs=1) as sp:
            d = sp.tile([P, 4, RCH, S], bf16, tag="d")
            nc.vector.tensor_tensor(d[:], PB[:, 0:4], MT4[:], op=ALU.subtract)
            ad = sp.tile([P, 4, RCH, S], bf16, tag="ad")
            nc.scalar.activation(ad[:], d[:], AF.Abs)                        # |d|
            tmh = sp.tile([P, 4, RCH, S], bf16, tag="tmh")
            nc.vector.tensor_scalar(tmh[:].rearrange("p q r s -> p (q r s)"),
                                    ad[:].rearrange("p q r s -> p (q r s)"),
                                    1.0, 0.5, op0=ALU.min, op1=ALU.mult)     # min(|d|,1)/2
            uu = sp.tile([P, 4, RCH, S], bf16, tag="uu")
            nc.vector.tensor_tensor(uu[:], ad[:], tmh[:], op=ALU.subtract)   # |d| - tm/2
            sl1h = sp.tile([P, 4, RCH, S], bf16, tag="sl1h")
            nc.vector.tensor_tensor(sl1h[:], tmh[:], uu[:], op=ALU.mult)     # sl1/2
            q2 = sp.tile([P, 2, RCH, S], bf16, tag="q2")
            nc.vector.tensor_tensor(q2[:], sl1h[:, 0:2], sl1h[:, 2:4], op=ALU.add)
            shs = sp.tile([P, RCH, S], bf16, tag="shs")
            nc.vector.tensor_tensor(shs[:], q2[:, 0], q2[:, 1], op=ALU.add)
            nc.scalar.activation(SL1T[:], shs[:].rearrange("p r s -> p s r"),
                                 AF.Copy, scale=2.0)                          # x2 fold

        # ---------- final match mask + weighted sums ----------
        EQB = sb.tile([P, RCH, S], bf16, tag="eqb")
        nc.vector.tensor_tensor(EQB[:], BEST[:],
                                BESTS16[:].unsqueeze(1).broadcast_to([P, RCH, S]), op=ALU.is_equal)
        M2 = sb.tile([P, RCH, S], bf16, tag="m2")
        nc.vector.tensor_tensor(M2[:], EQB[:],
                                ECN[:].unsqueeze(1).broadcast_to([P, RCH, S]), op=ALU.mult)
        MM = sb.tile([P, RCH, S], bf16, tag="mm")
        nc.vector.tensor_tensor(MM[:], MR[:], M2[:], op=ALU.add)

        FQ = sb.tile([P, 6, S, RCH], bf16, tag="fq")
        nc.scalar.copy(FQ[:, 0], MM[:].rearrange("p r s -> p s r"))
        nc.vector.tensor_tensor(FQ[:, 1], FQ[:, 0], SL1T[:], op=ALU.mult)
        nc.vector.tensor_tensor(FQ[:, 3], FQ[:, 0], SPN[:], op=ALU.mult)
        nc.vector.tensor_tensor(FQ[:, 4], FQ[:, 0], SPP[:], op=ALU.mult)
        nc.vector.tensor_copy(FQ[:, 5], SPP[:])

        # ---------- partition sums via ones-matmul (q=2 deferred until CE) ----------
        R768 = sb.tile([1, 6, S, RCH], f32, tag="r768")
        fqf = FQ[:].rearrange("p q s r -> p (q s r)")
        psr = ctx.enter_context(tc.tile_pool(name="psr", bufs=1, space="PSUM"))
        for lo, hi in ((0, 256), (384, 768)):
            rq_ps = psr.tile([1, 384], f32, tag="rq_ps", bufs=3, name=f"rq{lo}")
            nc.tensor.matmul(rq_ps[:, 0:hi - lo], ONESB[:], fqf[:, lo:hi], start=True, stop=True)
            nc.vector.tensor_copy(R768[:].rearrange("o q s r -> o (q s r)")[:, lo:hi], rq_ps[:, 0:hi - lo])

        # per-sample scalars that don't depend on CE: compute before CE finishes
        RQ = sb.tile([1, 6, S], f32, tag="rq")
        for q in (0, 1, 3, 4, 5):
            nc.vector.tensor_reduce(RQ[:, q], R768[:, q], axis=AX.X, op=ALU.add)
        mcnt = RQ[:, 0]; bbox_n = RQ[:, 1]
        spn_n = RQ[:, 3]; spp_m = RQ[:, 4]; spp_all = RQ[:, 5]

        def t16(tag):
            return sb.tile([1, S], f32, tag=tag, name=tag)

        d4 = t16("d4"); nc.vector.tensor_scalar(d4[:], mcnt, 4.0, 1.0, op0=ALU.mult, op1=ALU.max)
        r4 = t16("r4"); nc.vector.reciprocal(r4[:], d4[:])
        bbox = t16("bbox"); nc.vector.tensor_tensor(bbox[:], bbox_n, r4[:], op=ALU.mult)
        d1 = t16("d1"); nc.vector.tensor_scalar(d1[:], mcnt, 1.0, None, op0=ALU.max)
        r1 = t16("r1"); nc.vector.reciprocal(r1[:], d1[:])
        confm = t16("confm"); nc.vector.tensor_tensor(confm[:], spn_n, r1[:], op=ALU.mult)
        ucnt = t16("ucnt"); nc.vector.tensor_scalar(ucnt[:], mcnt, -1.0, float(N), op0=ALU.mult, op1=ALU.add)
        du = t16("du"); nc.vector.tensor_scalar(du[:], ucnt[:], 1.0, None, op0=ALU.max)
        ru = t16("ru"); nc.vector.reciprocal(ru[:], du[:])
        cun = t16("cun"); nc.vector.tensor_tensor(cun[:], spp_all, spp_m, op=ALU.subtract)
        confu = t16("confu"); nc.vector.tensor_tensor(confu[:], cun[:], ru[:], op=ALU.mult)
        csum = t16("csum"); nc.vector.tensor_tensor(csum[:], confm[:], confu[:], op=ALU.add)
        chalf = t16("chalf"); nc.vector.tensor_scalar(chalf[:], csum[:], 0.5, None, op0=ALU.mult)
        ug = t16("ug"); nc.vector.tensor_scalar(ug[:], ucnt[:], 0.0, None, op0=ALU.is_gt)
        ugn = t16("ugn"); nc.vector.tensor_scalar(ugn[:], ucnt[:], 0.0, None, op0=ALU.is_le)
        c1 = t16("c1"); nc.vector.tensor_tensor(c1[:], chalf[:], ug[:], op=ALU.mult)
        c2 = t16("c2"); nc.vector.tensor_tensor(c2[:], confm[:], ugn[:], op=ALU.mult)
        confL = t16("confL"); nc.vector.tensor_tensor(confL[:], c1[:], c2[:], op=ALU.add)
        lnv = t16("lnv"); nc.vector.tensor_scalar(lnv[:], spp_all, 1.0 / float(N), None, op0=ALU.mult)
        kvg = t16("kvg"); nc.vector.tensor_scalar(kvg[:], KV16[:], 0.0, None, op0=ALU.is_gt)
        kvn = t16("kvn"); nc.vector.tensor_scalar(kvn[:], KV16[:], 0.0, None, op0=ALU.is_le)
        bc = t16("bc"); nc.vector.tensor_tensor(bc[:], bbox[:], confL[:], op=ALU.add)

        LL2 = sb.tile([P, 2, ROWS], f32, tag="ll2")
        nc.scalar.activation(LL2[:], SP2[:], AF.Ln)
        CET = sb.tile([P, S, RCH], bf16, tag="cet")
        nc.vector.tensor_tensor(CET[:], LL2[:, 0].rearrange("p (s r) -> p s r", s=S),
                                LL2[:, 1].rearrange("p (s r) -> p s r", s=S), op=ALU.subtract)

        nc.vector.tensor_tensor(FQ[:, 2], FQ[:, 0], CET[:], op=ALU.mult)
        rq_ps2 = psr.tile([1, 384], f32, tag="rq_ps", bufs=3, name="rq2")
        nc.tensor.matmul(rq_ps2[:, 0:128], ONESB[:], fqf[:, 256:384], start=True, stop=True)
        nc.vector.tensor_copy(R768[:].rearrange("o q s r -> o (q s r)")[:, 256:384], rq_ps2[:, 0:128])

        # ---------- final: CE-dependent tail ----------
        nc.vector.tensor_reduce(RQ[:, 2], R768[:, 2], axis=AX.X, op=ALU.add)
        cls_n = RQ[:, 2]
        clsl = t16("clsl"); nc.vector.tensor_tensor(clsl[:], cls_n, r1[:], op=ALU.mult)
        lv = t16("lv"); nc.vector.tensor_tensor(lv[:], bc[:], clsl[:], op=ALU.add)
        lA = t16("lA"); nc.vector.tensor_tensor(lA[:], lv[:], kvg[:], op=ALU.mult)
        lB = t16("lB"); nc.vector.tensor_tensor(lB[:], lnv[:], kvn[:], op=ALU.mult)
        LROW = t16("lrow"); nc.vector.tensor_tensor(LROW[:], lA[:], lB[:], op=ALU.add)
        nc.sync.dma_start(loss_d[:], LROW[:])

    lp.__exit__(None, None, None)
    return preds_d, tgts_d, loss_d


_NC_CACHE = {}


def get_nc():
    if "nc" not in _NC_CACHE:
        nc = bacc.Bacc("TRN2", target_bir_lowering=False, debug=False)
        build_kernel(nc)
        nc.compile()
        _NC_CACHE["nc"] = nc
    return _NC_CACHE["nc"]


def kernel(preds: np.ndarray, targets: np.ndarray) -> np.ndarray:
    from concourse.bass_utils import run_bass_kernel_spmd

    nc = get_nc()
    in_maps = []
    for c in range(NCORES):
        in_maps.append({
            "preds": np.ascontiguousarray(preds[c * S:(c + 1) * S], dtype=np.float32),
            "tgts": np.ascontiguousarray(targets[c * S:(c + 1) * S], dtype=np.float32),
        })
    res = run_bass_kernel_spmd(nc, in_maps, core_ids=list(range(NCORES)))
    per_sample = np.concatenate([res.results[c]["loss"].reshape(-1) for c in range(NCORES)])
    return np.float32(per_sample.sum() / B)
